# revision 8
# baseline (speedup 1.0000x reference)
"""BotRGCN on 8 TRN2 NeuronCores (Bass/Tile SPMD kernel), v2.

Strategy (graph/data parallel):
  - Nodes sharded across 8 cores (12500/core); edges grouped by destination
    core and 512-wide destination window-pair; 128-dim weights replicated.
  - Activations live TRANSPOSED on-chip: [feat(128 partitions), nodes(free)].
  - Per RGCN layer: the node features are all-gathered into a 2-segment HBM
    table (two pipelined sub-collectives of 50000 rows each); x[src] rows for
    local edges are fetched with dma_gather using SIGNED int16 row offsets
    from a mid-segment base (covers 50000 rows per segment); scatter-add into
    per-window-pair PSUM accumulators via one-hot matmuls:
        pagg[feat, 512] += gathered[edges, feat].T @ onehot[edges, 512]
    where onehot[e, (win&1)*256 + rel*128 + (dst&127)] = 1 (pure 0/1; built
    with a single-op is_equal on the vector engine).
  - The per-(dst,rel) mean normalization 1/cnt is folded into the PSUM->SBUF
    eviction as a tensor_tensor multiply against a host-precomputed
    [128, 512] 1/cnt tile per pair (streamed from HBM; shared by both layers).
  - Relation transform: W_r.T @ mean + Wroot.T @ xT + bias, per 128-window.

The module is one SPMD program: the per-(pair,segment) chunk counts are
compiled as the max over cores; per-core variation lives in the gather
offsets / one-hot keys / padding (key -1 => contributes nothing).
"""

import math
from contextlib import ExitStack

import numpy as np

import concourse.bacc as bacc
import concourse.bass as bass
import concourse.mybir as mybir
import concourse.tile as tile
from concourse import bass_utils
from concourse.masks import make_identity

F32 = mybir.dt.float32
BF16 = mybir.dt.bfloat16
I16 = mybir.dt.int16
SLOPE = 0.01
N_CORES = 8
NSEG = 2          # gather-table segments (signed int16 offsets span 50000)
MAX_CALL = 8      # chunk slots per dma_gather call (<=1024 idxs)


# ---------------------------------------------------------------------------
# Host-side preprocessing
# ---------------------------------------------------------------------------

def _preprocess(edge_index, edge_type, n_nodes, n_cores=N_CORES):
    src = np.asarray(edge_index[0], dtype=np.int64)
    dst = np.asarray(edge_index[1], dtype=np.int64)
    et = np.asarray(edge_type, dtype=np.int64)
    E = src.shape[0]
    npc = n_nodes // n_cores
    assert npc * n_cores == n_nodes
    half = npc // 2                      # nodes per core per sub-collective
    seg_rows = half * n_cores            # rows per table segment
    assert seg_rows <= 65534
    base = seg_rows // 2                 # gather base row within a segment
    nw = (npc + 127) // 128
    npairs = (nw + 1) // 2

    # mean weights: 1 / count(dst, rel) -> per-core per-pair [512] tiles
    segid = dst * 2 + et
    cnt = np.bincount(segid, minlength=2 * n_nodes).astype(np.float32)
    cntinv_full = 1.0 / np.maximum(cnt, 1.0)          # [2N]
    cntinv = np.ones((n_cores, npairs, 512), dtype=np.float32)
    for c in range(n_cores):
        for p in range(npairs):
            for wh in range(2):
                w = 2 * p + wh
                lo = w * 128
                n_w = min(128, npc - lo)
                if n_w <= 0:
                    continue
                nodes = c * npc + lo + np.arange(n_w)
                for r in range(2):
                    cntinv[c, p, wh * 256 + r * 128:
                           wh * 256 + r * 128 + n_w] = \
                        cntinv_full[nodes * 2 + r]

    # table row for node (c, l): h = l // half; row(in seg h) = c*half + l%half
    sc = src // npc
    sl = src - sc * npc
    seg_e = sl // half
    row = sc * half + (sl - seg_e * half)
    off_e = row - base                                # signed int16 offset

    core = dst // npc
    dstl = dst - core * npc
    win = dstl >> 7
    pair = win >> 1
    key = ((win & 1) * 256 + et * 128 + (dstl & 127)).astype(np.float32)

    # group by (core, pair, seg); chunk counts compiled as max over cores
    gid = (core * npairs + pair) * NSEG + seg_e
    counts = np.bincount(gid, minlength=n_cores * npairs * NSEG
                         ).reshape(n_cores, npairs, NSEG)
    kq = np.ceil(counts.max(axis=0) / 128).astype(np.int64)  # [npairs, NSEG]

    # slot list (pair-major), gather-call schedule (runs of <=MAX_CALL slots
    # within one segment)
    slots = []          # (pair, seg)
    pair_slots = [[] for _ in range(npairs)]
    for p in range(npairs):
        for s in range(NSEG):
            for _ in range(kq[p, s]):
                pair_slots[p].append((len(slots), s))
                slots.append((p, s))
    nslots = len(slots)
    calls = []          # (seg, slot_lo, n_sl)
    i = 0
    while i < nslots:
        s = slots[i][1]
        j = i
        while j < nslots and j - i < MAX_CALL and slots[j][1] == s:
            j += 1
        calls.append((s, i, j - i))
        i = j
    slot_call = np.zeros(nslots, dtype=np.int64)       # slot -> call idx
    slot_ccol = np.zeros(nslots, dtype=np.int64)       # slot -> col in call
    for ci, (s, lo, n) in enumerate(calls):
        slot_call[lo:lo + n] = ci
        slot_ccol[lo:lo + n] = np.arange(n)

    # place edges: per (core, pair, seg) sequentially into that group's slots
    order = np.argsort(gid, kind="stable")
    gid_s = gid[order]
    starts = np.zeros(n_cores * npairs * NSEG + 1, dtype=np.int64)
    np.cumsum(counts.reshape(-1), out=starts[1:])
    pos = np.arange(E, dtype=np.int64) - starts[gid_s]

    # slot base index of group (p, s): first slot of that (p,s) run
    slot_base = np.zeros((npairs, NSEG), dtype=np.int64)
    acc = 0
    for p in range(npairs):
        for s in range(NSEG):
            slot_base[p, s] = acc
            acc += kq[p, s]

    e_core = core[order]
    e_pair = pair[order]
    e_seg = seg_e[order]
    e_slot = slot_base[e_pair, e_seg] + (pos >> 7)
    e_p = pos & 127

    gidx = np.zeros((n_cores, nslots * 128), dtype=np.int16)
    keym = np.full((n_cores, 128, nslots), -1.0, dtype=np.float32)
    gidx[e_core, e_slot * 128 + e_p] = off_e[order].astype(np.int16)
    keym[e_core, e_p, e_slot] = key[order]

    # guard: the LAST idx of each call must be >= 0 (the gather ucode trims
    # trailing negatives at runtime). Pads are 0, so only calls that end with
    # a full chunk whose last edge has a negative offset need a swap.
    for c in range(n_cores):
        for (s, lo, n) in calls:
            a, b = lo * 128, (lo + n) * 128
            if gidx[c, b - 1] < 0:
                cand = np.nonzero(gidx[c, a:b] >= 0)[0]
                assert cand.size > 0, "gather call with all-negative offsets"
                j = a + cand[-1]
                f1, f2 = j, b - 1
                gidx[c, f1], gidx[c, f2] = gidx[c, f2], gidx[c, f1]
                p1, s1 = f1 & 127, f1 >> 7
                p2, s2 = f2 & 127, f2 >> 7
                tmp = keym[c, p1, s1]
                keym[c, p1, s1] = keym[c, p2, s2]
                keym[c, p2, s2] = tmp

    # wrap indices: position i -> [i%16, i//16], replicated to 128 partitions
    idx16 = np.ascontiguousarray(
        gidx.reshape(n_cores, nslots * 8, 16).transpose(0, 2, 1))
    idx16 = np.tile(idx16, (1, 8, 1))  # [n_cores, 128, nslots*8]

    cntinv_t = np.ascontiguousarray(
        np.broadcast_to(cntinv.reshape(n_cores, 1, npairs * 512),
                        (n_cores, 128, npairs * 512)))

    return dict(
        npc=npc, nw=nw, npairs=npairs, half=half, seg_rows=seg_rows,
        base=base, kq=kq, slots=slots, calls=calls, slot_call=slot_call,
        slot_ccol=slot_ccol, pair_slots=pair_slots, nslots=nslots,
        idx16=idx16, keym=keym, cntinv=cntinv_t,
    )


# ---------------------------------------------------------------------------
# Device kernel builder (one SPMD module for all cores)
# ---------------------------------------------------------------------------

def _build_module(N, T, prep, n_cores=N_CORES, gather_dtype="bfloat16"):
    D = 128
    KT = T // 128
    assert KT * 128 == T
    npc = prep["npc"]
    nw = prep["nw"]
    npairs = prep["npairs"]
    half = prep["half"]
    seg_rows = prep["seg_rows"]
    base = prep["base"]
    calls = prep["calls"]
    pair_slots = prep["pair_slots"]
    nslots = prep["nslots"]
    npad = nw * 128
    TILE_W = 512
    NT = (npc + TILE_W - 1) // TILE_W
    GDT = F32 if gather_dtype == "float32" else BF16

    nc = bacc.Bacc("TRN2", target_bir_lowering=False, debug=False,
                   enable_asserts=False, num_devices=n_cores)

    # ---- I/O -------------------------------------------------------------
    tweetT_d = nc.dram_tensor("tweetT", [T, npc], GDT, kind="ExternalInput")
    idx_d = nc.dram_tensor("idx16", [128, nslots * 8], I16,
                           kind="ExternalInput")
    keym_d = nc.dram_tensor("keym", [128, nslots], F32, kind="ExternalInput")
    cntinv_d = nc.dram_tensor("cntinv", [128, npairs * 512], F32,
                              kind="ExternalInput")
    Wt_d = nc.dram_tensor("Wt", [T, D], GDT, kind="ExternalInput")
    Wi_d = nc.dram_tensor("Wi", [D, D], F32, kind="ExternalInput")
    Wr0_d = nc.dram_tensor("Wr0", [D, D], F32, kind="ExternalInput")
    Wr1_d = nc.dram_tensor("Wr1", [D, D], F32, kind="ExternalInput")
    Wroot_d = nc.dram_tensor("Wroot", [D, D], F32, kind="ExternalInput")
    Wo_d = nc.dram_tensor("Wo", [D, D], F32, kind="ExternalInput")
    Wout_d = nc.dram_tensor("Wout", [D, 2], F32, kind="ExternalInput")
    bt_d = nc.dram_tensor("bt", [D, 1], F32, kind="ExternalInput")
    bi_d = nc.dram_tensor("bi", [D, 1], F32, kind="ExternalInput")
    brgcn_d = nc.dram_tensor("brgcn", [D, 1], F32, kind="ExternalInput")
    bo_d = nc.dram_tensor("bo", [D, 1], F32, kind="ExternalInput")
    bout_d = nc.dram_tensor("bout", [2, 1], F32, kind="ExternalInput")
    outT_d = nc.dram_tensor("outT", [2, npc], F32, kind="ExternalOutput")

    # all-gather outputs: 2 segments per layer
    xf1 = [nc.dram_tensor(f"xfull1_{k}", [seg_rows, 128], GDT)
           for k in range(NSEG)]
    xf2 = [nc.dram_tensor(f"xfull2_{k}", [seg_rows, 128], GDT)
           for k in range(NSEG)]

    rg = [list(range(n_cores))]

    with tile.TileContext(nc) as tc, ExitStack() as ctx:
        # ---- persistent SBUF state --------------------------------------
        wpool = ctx.enter_context(tc.tile_pool(name="wpool", bufs=1))
        wt_sb = wpool.tile([128, KT * 128], GDT)
        for k in range(KT):
            nc.sync.dma_start(out=wt_sb[:, k * 128:(k + 1) * 128],
                              in_=Wt_d[k * 128:(k + 1) * 128, :])
        wi_sb = wpool.tile([128, 128], F32)
        nc.sync.dma_start(out=wi_sb[:], in_=Wi_d[:, :])
        wr0_sb = wpool.tile([128, 128], F32)
        nc.sync.dma_start(out=wr0_sb[:], in_=Wr0_d[:, :])
        wr1_sb = wpool.tile([128, 128], F32)
        nc.sync.dma_start(out=wr1_sb[:], in_=Wr1_d[:, :])
        wroot_sb = wpool.tile([128, 128], F32)
        nc.sync.dma_start(out=wroot_sb[:], in_=Wroot_d[:, :])
        wo_sb = wpool.tile([128, 128], F32)
        nc.sync.dma_start(out=wo_sb[:], in_=Wo_d[:, :])
        wout_sb = wpool.tile([128, 2], F32)
        nc.sync.dma_start(out=wout_sb[:], in_=Wout_d[:, :])
        bt_sb = wpool.tile([128, 1], F32)
        nc.sync.dma_start(out=bt_sb[:], in_=bt_d[:, :])
        bi_sb = wpool.tile([128, 1], F32)
        nc.sync.dma_start(out=bi_sb[:], in_=bi_d[:, :])
        brgcn_sb = wpool.tile([128, 1], F32)
        nc.sync.dma_start(out=brgcn_sb[:], in_=brgcn_d[:, :])
        bo_sb = wpool.tile([128, 1], F32)
        nc.sync.dma_start(out=bo_sb[:], in_=bo_d[:, :])
        bout_sb = wpool.tile([2, 1], F32)
        nc.sync.dma_start(out=bout_sb[:], in_=bout_d[:, :])

        idx_sb = wpool.tile([128, nslots * 8], I16)
        nc.sync.dma_start(out=idx_sb[:], in_=idx_d[:, :])
        keym_sb = wpool.tile([128, nslots], F32)
        nc.sync.dma_start(out=keym_sb[:], in_=keym_d[:, :])

        iota_sb = wpool.tile([128, 512], I16)
        nc.gpsimd.iota(iota_sb[:], pattern=[[1, 512]], base=0,
                       channel_multiplier=0,
                       allow_small_or_imprecise_dtypes=True)
        ident_sb = wpool.tile([128, 128], F32)
        make_identity(nc, ident_sb[:])

        # persistent transposed activations (xa reused for layer-2 output)
        xa = wpool.tile([128, npad], F32)   # x1T, later x3T
        xb = wpool.tile([128, npad], F32)   # x2T
        if npad > npc:
            nc.vector.memset(xa[:, npc:npad], 0.0)
            nc.vector.memset(xb[:, npc:npad], 0.0)

        # DRAM staging for the all-gather inputs
        dpool = ctx.enter_context(tc.tile_pool(name="dpool", bufs=1,
                                               space="DRAM"))
        ag1_in = dpool.tile([npc, 128], GDT)
        ag2_in = dpool.tile([npc, 128], GDT)

        # ---- helpers ----------------------------------------------------
        def leaky_inplace(ap):
            nc.vector.scalar_tensor_tensor(out=ap, in0=ap, scalar=SLOPE,
                                           in1=ap, op0=mybir.AluOpType.mult,
                                           op1=mybir.AluOpType.max)

        def transpose_to_nat(src_slice, w, nat_pool, tp_pool, ag_in):
            # src_slice: [128 feat, 128 nodes] slice of an xT tile
            ptp = tp_pool.tile([128, 128], F32, name="ptp")
            nc.tensor.transpose(ptp[:], src_slice, ident_sb[:])
            nat = nat_pool.tile([128, 128], GDT, name="nat")
            nc.scalar.activation(out=nat[:], in_=ptp[:],
                                 func=mybir.ActivationFunctionType.Copy)
            wsz = min(128, npc - w * 128)
            nc.sync.dma_start(out=ag_in[w * 128: w * 128 + wsz, :],
                              in_=nat[:wsz, :])

        def emit_subag(ag_in, xfs, k):
            nc.gpsimd.collective_compute(
                "AllGather", mybir.AluOpType.bypass, replica_groups=rg,
                ins=[ag_in[k * half:(k + 1) * half, :]],
                outs=[xfs[k][:, :]])

        # ---- stage 1: x1 = leaky(tweet @ Wt + bt); leaky(x1 @ Wi + bi) --
        with tc.tile_pool(name="s1psum", bufs=2, space="PSUM") as s1psum, \
             tc.tile_pool(name="s1psum2", bufs=2, space="PSUM") as s1psum2, \
             tc.tile_pool(name="s1buf", bufs=3) as s1buf, \
             tc.tile_pool(name="s1nat", bufs=3) as s1nat, \
             tc.tile_pool(name="s1tp", bufs=2, space="PSUM") as s1tp:
            for t in range(NT):
                c0 = t * TILE_W
                cw = min(TILE_W, npc - c0)
                ps1 = s1psum.tile([128, TILE_W], F32, name="ps1")
                for k in range(KT):
                    tw = s1buf.tile([128, TILE_W], GDT, name="tw")
                    nc.sync.dma_start(
                        out=tw[:, :cw],
                        in_=tweetT_d[k * 128:(k + 1) * 128, c0:c0 + cw])
                    nc.tensor.matmul(ps1[:, :cw],
                                     lhsT=wt_sb[:, k * 128:(k + 1) * 128],
                                     rhs=tw[:, :cw],
                                     start=(k == 0), stop=(k == KT - 1))
                x1b = s1buf.tile([128, TILE_W], F32, name="x1b")
                nc.scalar.activation(out=x1b[:, :cw], in_=ps1[:, :cw],
                                     func=mybir.ActivationFunctionType.Lrelu,
                                     bias=bt_sb[:, :1], alpha=SLOPE)
                ps2 = s1psum2.tile([128, TILE_W], F32, name="ps2")
                nc.tensor.matmul(ps2[:, :cw], lhsT=wi_sb[:], rhs=x1b[:, :cw],
                                 start=True, stop=True)
                nc.vector.tensor_scalar(out=xa[:, c0:c0 + cw],
                                        in0=ps2[:, :cw],
                                        scalar1=bi_sb[:, :1], scalar2=None,
                                        op0=mybir.AluOpType.add)
                leaky_inplace(xa[:, c0:c0 + cw])
                for wi_ in range(c0 // 128, (c0 + cw + 127) // 128):
                    transpose_to_nat(xa[:, wi_ * 128:(wi_ + 1) * 128], wi_,
                                     s1nat, s1tp, ag1_in)
                # fire sub-collective 0 as soon as its half is produced
                if c0 < half <= c0 + cw:
                    emit_subag(ag1_in, xf1, 0)
            emit_subag(ag1_in, xf1, 1)

        # ---- RGCN layers -------------------------------------------------
        # pair after which each next-layer sub-collective can fire:
        # sub-ag k needs nat rows [k*half, (k+1)*half) = pairs up to
        # ceil((k+1)*half/256)-1
        subag_after = [(1 * half + 255) // 256 - 1, npairs - 1]

        def rgcn_layer(xin, xout, xfs, ag_next, xfs_next):
            with tc.tile_pool(name="stagp", bufs=6) as stagp, \
                 tc.tile_pool(name="mp", bufs=12) as mp, \
                 tc.tile_pool(name="aggp", bufs=4, space="PSUM") as aggp, \
                 tc.tile_pool(name="meanp", bufs=3) as meanp, \
                 tc.tile_pool(name="cip", bufs=3) as cip, \
                 tc.tile_pool(name="trp", bufs=2, space="PSUM") as trp, \
                 tc.tile_pool(name="tpp", bufs=2, space="PSUM") as tpp, \
                 tc.tile_pool(name="natp", bufs=3) as natp:
                stag_tiles = {}

                def emit_gather(ci):
                    s, lo, n = calls[ci]
                    st = stagp.tile([128, MAX_CALL * 128], GDT, name="st")
                    stag_tiles[ci] = st
                    n_i = n * 128
                    nc.gpsimd.dma_gather(
                        out_ap=st[:, :n_i].rearrange("p (c d) -> p c d",
                                                     d=128),
                        in_ap=xfs[s][base:seg_rows, :],
                        idxs_ap=idx_sb[:, lo * 8: lo * 8 + n_i // 16],
                        num_idxs=n_i,
                        num_idxs_reg=n_i,
                        elem_size=128,
                    )

                def emit_tail(p, pagg, ci_t):
                    # pair tail, emitted one pair late (software pipeline):
                    # mean (DVE), relation transform (PE), bias (ScalarE),
                    # transpose-to-nat (PE + ScalarE), next-layer sub-ags
                    mean = meanp.tile([128, 512], F32, name="mean")
                    nc.vector.tensor_tensor(out=mean[:], in0=pagg[:],
                                            in1=ci_t[:],
                                            op=mybir.AluOpType.mult)
                    ptr = trp.tile([128, 256], F32, name="ptr")
                    for wh in range(2):
                        w = p * 2 + wh
                        if w >= nw:
                            nc.vector.memset(ptr[:, wh * 128:(wh + 1) * 128],
                                             0.0)
                            continue
                        po = ptr[:, wh * 128:(wh + 1) * 128]
                        nc.tensor.matmul(
                            po, lhsT=wr0_sb[:],
                            rhs=mean[:, wh * 256:wh * 256 + 128],
                            start=True, stop=False)
                        nc.tensor.matmul(
                            po, lhsT=wr1_sb[:],
                            rhs=mean[:, wh * 256 + 128:wh * 256 + 256],
                            start=False, stop=False)
                        nc.tensor.matmul(
                            po, lhsT=wroot_sb[:],
                            rhs=xin[:, w * 128:(w + 1) * 128],
                            start=False, stop=True)
                    psz = min(256, npad - p * 256)
                    nc.scalar.activation(
                        out=xout[:, p * 256:p * 256 + psz],
                        in_=ptr[:, :psz],
                        func=mybir.ActivationFunctionType.Identity,
                        bias=brgcn_sb[:, :1])
                    if ag_next is not None:
                        for wh in range(2):
                            w = p * 2 + wh
                            if w >= nw:
                                continue
                            transpose_to_nat(xout[:, w * 128:(w + 1) * 128],
                                             w, natp, tpp, ag_next)
                        if p == subag_after[0]:
                            emit_subag(ag_next, xfs_next, 0)
                        elif p == subag_after[1]:
                            emit_subag(ag_next, xfs_next, 1)

                next_call = 0
                pending = None
                for p in range(npairs):
                    psl = pair_slots[p]
                    # make sure all gathers covering this pair are emitted
                    last_call = prep["slot_call"][psl[-1][0]]
                    while next_call <= last_call:
                        emit_gather(next_call)
                        next_call += 1
                    ci_t = cip.tile([128, 512], F32, name="ci")
                    nc.sync.dma_start(out=ci_t[:],
                                      in_=cntinv_d[:, p * 512:(p + 1) * 512])
                    pagg = aggp.tile([128, 512], F32, name="pagg")
                    nmm = len(psl)
                    for i, (sl, s) in enumerate(psl):
                        ci = prep["slot_call"][sl]
                        col = prep["slot_ccol"][sl]
                        st = stag_tiles[ci]
                        m = mp.tile([128, 512], GDT, name="m")
                        nc.vector.tensor_scalar(
                            out=m[:], in0=iota_sb[:],
                            scalar1=keym_sb[:, sl:sl + 1], scalar2=None,
                            op0=mybir.AluOpType.is_equal)
                        nc.tensor.matmul(
                            pagg[:],
                            lhsT=st[:, col * 128:(col + 1) * 128],
                            rhs=m[:],
                            start=(i == 0), stop=(i == nmm - 1))
                    if pending is not None:
                        emit_tail(*pending)
                    pending = (p, pagg, ci_t)
                emit_tail(*pending)

        rgcn_layer(xa, xb, xf1, ag2_in, xf2)
        rgcn_layer(xb, xa, xf2, None, None)

        # ---- head: leaky(x @ Wo + bo) @ Wout + bout ---------------------
        with tc.tile_pool(name="hps", bufs=2, space="PSUM") as hps, \
             tc.tile_pool(name="hps2", bufs=2, space="PSUM") as hps2, \
             tc.tile_pool(name="hbuf", bufs=3) as hbuf:
            for t in range(NT):
                c0 = t * TILE_W
                cw = min(TILE_W, npc - c0)
                psh = hps.tile([128, TILE_W], F32, name="psh")
                nc.tensor.matmul(psh[:, :cw], lhsT=wo_sb[:],
                                 rhs=xa[:, c0:c0 + cw], start=True, stop=True)
                hb = hbuf.tile([128, TILE_W], F32, name="hb")
                nc.scalar.activation(out=hb[:, :cw], in_=psh[:, :cw],
                                     func=mybir.ActivationFunctionType.Lrelu,
                                     bias=bo_sb[:, :1], alpha=SLOPE)
                pso = hps2.tile([2, TILE_W], F32, name="pso")
                nc.tensor.matmul(pso[:, :cw], lhsT=wout_sb[:],
                                 rhs=hb[:, :cw], start=True, stop=True)
                ob = hbuf.tile([2, TILE_W], F32, name="ob")
                nc.vector.tensor_scalar(out=ob[:, :cw], in0=pso[:, :cw],
                                        scalar1=bout_sb[:, :1], scalar2=None,
                                        op0=mybir.AluOpType.add)
                nc.sync.dma_start(out=outT_d[:, c0:c0 + cw], in_=ob[:, :cw])

    nc.compile()
    return nc


# ---------------------------------------------------------------------------
# Public entry point
# ---------------------------------------------------------------------------

_CACHE = {}
GATHER_DTYPE = "bfloat16"


def _get_module(N, T, prep, gather_dtype=None):
    if gather_dtype is None:
        gather_dtype = GATHER_DTYPE
    key = (N, T, prep["npc"], prep["nslots"], tuple(prep["kq"].reshape(-1)),
           gather_dtype)
    if key not in _CACHE:
        _CACHE[key] = _build_module(N, T, prep, gather_dtype=gather_dtype)
    return _CACHE[key]


def _make_in_maps(tweet, prep, Wt, bt, Wi, bi, Wrel, Wroot, brgcn, Wo, bo,
                  Wout, bout, n_cores=N_CORES, gather_dtype=None):
    import ml_dtypes
    if gather_dtype is None:
        gather_dtype = GATHER_DTYPE
    gdt = np.float32 if gather_dtype == "float32" else ml_dtypes.bfloat16
    npc = prep["npc"]
    f32 = np.float32
    shared = dict(
        Wt=np.ascontiguousarray(np.asarray(Wt, f32).astype(gdt)),
        Wi=np.ascontiguousarray(Wi, f32),
        Wr0=np.ascontiguousarray(Wrel[0], f32),
        Wr1=np.ascontiguousarray(Wrel[1], f32),
        Wroot=np.ascontiguousarray(Wroot, f32),
        Wo=np.ascontiguousarray(Wo, f32),
        Wout=np.ascontiguousarray(Wout, f32),
        bt=np.ascontiguousarray(np.reshape(bt, (-1, 1)), f32),
        bi=np.ascontiguousarray(np.reshape(bi, (-1, 1)), f32),
        brgcn=np.ascontiguousarray(np.reshape(brgcn, (-1, 1)), f32),
        bo=np.ascontiguousarray(np.reshape(bo, (-1, 1)), f32),
        bout=np.ascontiguousarray(np.reshape(bout, (-1, 1)), f32),
    )
    in_maps = []
    for c in range(n_cores):
        m = dict(shared)
        m["tweetT"] = np.ascontiguousarray(
            tweet[c * npc:(c + 1) * npc].T.astype(gdt))
        m["idx16"] = np.ascontiguousarray(prep["idx16"][c])
        m["keym"] = np.ascontiguousarray(prep["keym"][c])
        m["cntinv"] = np.ascontiguousarray(prep["cntinv"][c])
        in_maps.append(m)
    return in_maps


def kernel(tweet, edge_index, edge_type, Wt, bt, Wi, bi, Wrel, Wroot, brgcn,
           Wo, bo, Wout, bout):
    tweet = np.asarray(tweet, dtype=np.float32)
    N, T = tweet.shape
    prep = _preprocess(edge_index, edge_type, N)
    nc = _get_module(N, T, prep)
    in_maps = _make_in_maps(tweet, prep, Wt, bt, Wi, bi, Wrel, Wroot, brgcn,
                            Wo, bo, Wout, bout)
    res = bass_utils.run_bass_kernel_spmd(
        nc, in_maps, core_ids=list(range(N_CORES)))
    out = np.concatenate(
        [res.results[c]["outT"].T for c in range(N_CORES)], axis=0)
    return np.ascontiguousarray(out, dtype=np.float32)


# revision 21
# speedup vs baseline: 1.0157x; 1.0157x over previous
"""BotRGCN on 8 TRN2 NeuronCores (Bass/Tile SPMD kernel), v2.

Strategy (graph/data parallel):
  - Nodes sharded across 8 cores (12500/core); edges grouped by destination
    core and 512-wide destination window-pair; 128-dim weights replicated.
  - Activations live TRANSPOSED on-chip: [feat(128 partitions), nodes(free)].
  - Per RGCN layer: the node features are all-gathered into a 2-segment HBM
    table (two pipelined sub-collectives of 50000 rows each); x[src] rows for
    local edges are fetched with dma_gather using SIGNED int16 row offsets
    from a mid-segment base (covers 50000 rows per segment); scatter-add into
    per-window-pair PSUM accumulators via one-hot matmuls:
        pagg[feat, 512] += gathered[edges, feat].T @ onehot[edges, 512]
    where onehot[e, (win&1)*256 + rel*128 + (dst&127)] = 1 (pure 0/1; built
    with a single-op is_equal on the vector engine).
  - The per-(dst,rel) mean normalization 1/cnt is folded into the PSUM->SBUF
    eviction as a tensor_tensor multiply against a host-precomputed
    [128, 512] 1/cnt tile per pair (streamed from HBM; shared by both layers).
  - Relation transform: W_r.T @ mean + Wroot.T @ xT + bias, per 128-window.

The module is one SPMD program: the per-(pair,segment) chunk counts are
compiled as the max over cores; per-core variation lives in the gather
offsets / one-hot keys / padding (key -1 => contributes nothing).
"""

import math
from contextlib import ExitStack

import numpy as np

import concourse.bacc as bacc
import concourse.bass as bass
import concourse.mybir as mybir
import concourse.tile as tile
from concourse import bass_utils
from concourse.masks import make_identity

F32 = mybir.dt.float32
BF16 = mybir.dt.bfloat16
I16 = mybir.dt.int16
SLOPE = 0.01
N_CORES = 8
NSEG = 2          # gather-table segments (signed int16 offsets span 50000)
MAX_CALL = 8      # chunk slots per dma_gather call (<=1024 idxs)


# ---------------------------------------------------------------------------
# Host-side preprocessing
# ---------------------------------------------------------------------------

def _preprocess(edge_index, edge_type, n_nodes, n_cores=N_CORES):
    src = np.asarray(edge_index[0], dtype=np.int64)
    dst = np.asarray(edge_index[1], dtype=np.int64)
    et = np.asarray(edge_type, dtype=np.int64)
    E = src.shape[0]
    npc = n_nodes // n_cores
    assert npc * n_cores == n_nodes
    half = npc // 2                      # nodes per core per sub-collective
    seg_rows = half * n_cores            # rows per table segment
    assert seg_rows <= 65534
    base = seg_rows // 2                 # gather base row within a segment
    nw = (npc + 127) // 128
    npairs = (nw + 1) // 2

    # mean weights: 1 / count(dst, rel) -> per-core per-pair [512] tiles
    segid = dst * 2 + et
    cnt = np.bincount(segid, minlength=2 * n_nodes).astype(np.float32)
    cntinv_full = 1.0 / np.maximum(cnt, 1.0)          # [2N]
    cntinv = np.ones((n_cores, npairs, 512), dtype=np.float32)
    for c in range(n_cores):
        for p in range(npairs):
            for wh in range(2):
                w = 2 * p + wh
                lo = w * 128
                n_w = min(128, npc - lo)
                if n_w <= 0:
                    continue
                nodes = c * npc + lo + np.arange(n_w)
                for r in range(2):
                    cntinv[c, p, wh * 256 + r * 128:
                           wh * 256 + r * 128 + n_w] = \
                        cntinv_full[nodes * 2 + r]

    # table row for node (c, l): quarter q = l // quart; global row =
    # q*(n_cores*quart) + c*quart + l%quart  (quarters are contiguous
    # sub-collective outputs; segment s = q//2 spans 2 quarters)
    quart = npc // 4
    rows_sub = n_cores * quart
    sc = src // npc
    sl = src - sc * npc
    q_e = sl // quart
    grow = q_e * rows_sub + sc * quart + (sl - q_e * quart)
    seg_e = grow // seg_rows
    off_e = (grow - seg_e * seg_rows) - base          # signed int16 offset

    core = dst // npc
    dstl = dst - core * npc
    win = dstl >> 7
    pair = win >> 1
    key = ((win & 1) * 256 + et * 128 + (dstl & 127)).astype(np.float32)

    # group by (core, pair, seg); chunk counts compiled as max over cores
    gid = (core * npairs + pair) * NSEG + seg_e
    counts = np.bincount(gid, minlength=n_cores * npairs * NSEG
                         ).reshape(n_cores, npairs, NSEG)
    kq = np.ceil(counts.max(axis=0) / 128).astype(np.int64)  # [npairs, NSEG]

    # slot list (pair-major), gather-call schedule (runs of <=MAX_CALL slots
    # within one segment)
    slots = []          # (pair, seg)
    pair_slots = [[] for _ in range(npairs)]
    for p in range(npairs):
        for s in range(NSEG):
            for _ in range(kq[p, s]):
                pair_slots[p].append((len(slots), s))
                slots.append((p, s))
    nslots = len(slots)
    calls = []          # (seg, slot_lo, n_sl)
    i = 0
    while i < nslots:
        s = slots[i][1]
        j = i
        while j < nslots and j - i < MAX_CALL and slots[j][1] == s:
            j += 1
        calls.append((s, i, j - i))
        i = j
    slot_call = np.zeros(nslots, dtype=np.int64)       # slot -> call idx
    slot_ccol = np.zeros(nslots, dtype=np.int64)       # slot -> col in call
    for ci, (s, lo, n) in enumerate(calls):
        slot_call[lo:lo + n] = ci
        slot_ccol[lo:lo + n] = np.arange(n)

    # gather-call emission schedule: seg-0 calls are emitted LEAD pairs
    # ahead of their first consumer so they stream while the later
    # sub-collectives (needed by seg-1) are still in flight.
    LEAD = 5
    sched = [[] for _ in range(npairs)]
    for ci, (s, lo, n) in enumerate(calls):
        first_pair = slots[lo][0]
        step = max(0, first_pair - (LEAD if s == 0 else 0))
        sched[step].append(ci)
    for step in range(npairs):       # seg-1 (just-in-time) first
        sched[step].sort(key=lambda ci: -calls[ci][0])

    # place edges: per (core, pair, seg) sequentially into that group's slots
    order = np.argsort(gid, kind="stable")
    gid_s = gid[order]
    starts = np.zeros(n_cores * npairs * NSEG + 1, dtype=np.int64)
    np.cumsum(counts.reshape(-1), out=starts[1:])
    pos = np.arange(E, dtype=np.int64) - starts[gid_s]

    # slot base index of group (p, s): first slot of that (p,s) run
    slot_base = np.zeros((npairs, NSEG), dtype=np.int64)
    acc = 0
    for p in range(npairs):
        for s in range(NSEG):
            slot_base[p, s] = acc
            acc += kq[p, s]

    e_core = core[order]
    e_pair = pair[order]
    e_seg = seg_e[order]
    e_slot = slot_base[e_pair, e_seg] + (pos >> 7)
    e_p = pos & 127

    gidx = np.zeros((n_cores, nslots * 128), dtype=np.int16)
    keym = np.full((n_cores, 128, nslots), -1.0, dtype=np.float32)
    gidx[e_core, e_slot * 128 + e_p] = off_e[order].astype(np.int16)
    keym[e_core, e_p, e_slot] = key[order]

    # guard: the LAST idx of each call must be >= 0 (the gather ucode trims
    # trailing negatives at runtime). Pads are 0, so only calls that end with
    # a full chunk whose last edge has a negative offset need a swap.
    for c in range(n_cores):
        for (s, lo, n) in calls:
            a, b = lo * 128, (lo + n) * 128
            if gidx[c, b - 1] < 0:
                cand = np.nonzero(gidx[c, a:b] >= 0)[0]
                assert cand.size > 0, "gather call with all-negative offsets"
                j = a + cand[-1]
                f1, f2 = j, b - 1
                gidx[c, f1], gidx[c, f2] = gidx[c, f2], gidx[c, f1]
                p1, s1 = f1 & 127, f1 >> 7
                p2, s2 = f2 & 127, f2 >> 7
                tmp = keym[c, p1, s1]
                keym[c, p1, s1] = keym[c, p2, s2]
                keym[c, p2, s2] = tmp

    # wrap indices: position i -> [i%16, i//16], replicated to 128 partitions
    idx16 = np.ascontiguousarray(
        gidx.reshape(n_cores, nslots * 8, 16).transpose(0, 2, 1))
    idx16 = np.tile(idx16, (1, 8, 1))  # [n_cores, 128, nslots*8]

    cntinv_t = np.ascontiguousarray(
        np.broadcast_to(cntinv.reshape(n_cores, 1, npairs * 512),
                        (n_cores, 128, npairs * 512)))

    return dict(
        npc=npc, nw=nw, npairs=npairs, half=half, seg_rows=seg_rows,
        base=base, kq=kq, slots=slots, calls=calls, slot_call=slot_call,
        slot_ccol=slot_ccol, pair_slots=pair_slots, nslots=nslots,
        sched=sched, idx16=idx16, keym=keym, cntinv=cntinv_t,
    )


# ---------------------------------------------------------------------------
# Device kernel builder (one SPMD module for all cores)
# ---------------------------------------------------------------------------

def _build_module(N, T, prep, n_cores=N_CORES, gather_dtype="bfloat16"):
    D = 128
    KT = T // 128
    assert KT * 128 == T
    npc = prep["npc"]
    nw = prep["nw"]
    npairs = prep["npairs"]
    half = prep["half"]
    seg_rows = prep["seg_rows"]
    base = prep["base"]
    calls = prep["calls"]
    pair_slots = prep["pair_slots"]
    nslots = prep["nslots"]
    npad = nw * 128
    TILE_W = 512
    NT = (npc + TILE_W - 1) // TILE_W
    GDT = F32 if gather_dtype == "float32" else BF16

    nc = bacc.Bacc("TRN2", target_bir_lowering=False, debug=False,
                   enable_asserts=False, num_devices=n_cores)

    # ---- I/O -------------------------------------------------------------
    tweetT_d = nc.dram_tensor("tweetT", [T, npc], GDT, kind="ExternalInput")
    idx_d = nc.dram_tensor("idx16", [128, nslots * 8], I16,
                           kind="ExternalInput")
    keym_d = nc.dram_tensor("keym", [128, nslots], F32, kind="ExternalInput")
    cntinv_d = nc.dram_tensor("cntinv", [128, npairs * 512], F32,
                              kind="ExternalInput")
    Wt_d = nc.dram_tensor("Wt", [T, D], GDT, kind="ExternalInput")
    Wi_d = nc.dram_tensor("Wi", [D, D], F32, kind="ExternalInput")
    Wr0_d = nc.dram_tensor("Wr0", [D, D], F32, kind="ExternalInput")
    Wr1_d = nc.dram_tensor("Wr1", [D, D], F32, kind="ExternalInput")
    Wroot_d = nc.dram_tensor("Wroot", [D, D], F32, kind="ExternalInput")
    Wo_d = nc.dram_tensor("Wo", [D, D], F32, kind="ExternalInput")
    Wout_d = nc.dram_tensor("Wout", [D, 2], F32, kind="ExternalInput")
    bt_d = nc.dram_tensor("bt", [D, 1], F32, kind="ExternalInput")
    bi_d = nc.dram_tensor("bi", [D, 1], F32, kind="ExternalInput")
    brgcn_d = nc.dram_tensor("brgcn", [D, 1], F32, kind="ExternalInput")
    bo_d = nc.dram_tensor("bo", [D, 1], F32, kind="ExternalInput")
    bout_d = nc.dram_tensor("bout", [2, 1], F32, kind="ExternalInput")
    outT_d = nc.dram_tensor("outT", [2, npc], F32, kind="ExternalOutput")

    # all-gather output tables (4 contiguous quarter-collectives each)
    xf1 = nc.dram_tensor("xfull1", [N, 128], GDT)
    xf2 = nc.dram_tensor("xfull2", [N, 128], GDT)

    rg = [list(range(n_cores))]

    with tile.TileContext(nc) as tc, ExitStack() as ctx:
        # ---- persistent SBUF state --------------------------------------
        wpool = ctx.enter_context(tc.tile_pool(name="wpool", bufs=1))
        wt_sb = wpool.tile([128, KT * 128], GDT)
        for k in range(KT):
            nc.sync.dma_start(out=wt_sb[:, k * 128:(k + 1) * 128],
                              in_=Wt_d[k * 128:(k + 1) * 128, :])
        wi_sb = wpool.tile([128, 128], F32)
        nc.sync.dma_start(out=wi_sb[:], in_=Wi_d[:, :])
        wr0_sb = wpool.tile([128, 128], F32)
        nc.sync.dma_start(out=wr0_sb[:], in_=Wr0_d[:, :])
        wr1_sb = wpool.tile([128, 128], F32)
        nc.sync.dma_start(out=wr1_sb[:], in_=Wr1_d[:, :])
        wroot_sb = wpool.tile([128, 128], F32)
        nc.sync.dma_start(out=wroot_sb[:], in_=Wroot_d[:, :])
        wo_sb = wpool.tile([128, 128], F32)
        nc.sync.dma_start(out=wo_sb[:], in_=Wo_d[:, :])
        wout_sb = wpool.tile([128, 2], F32)
        nc.sync.dma_start(out=wout_sb[:], in_=Wout_d[:, :])
        bt_sb = wpool.tile([128, 1], F32)
        nc.sync.dma_start(out=bt_sb[:], in_=bt_d[:, :])
        bi_sb = wpool.tile([128, 1], F32)
        nc.sync.dma_start(out=bi_sb[:], in_=bi_d[:, :])
        brgcn_sb = wpool.tile([128, 1], F32)
        nc.sync.dma_start(out=brgcn_sb[:], in_=brgcn_d[:, :])
        bo_sb = wpool.tile([128, 1], F32)
        nc.sync.dma_start(out=bo_sb[:], in_=bo_d[:, :])
        bout_sb = wpool.tile([2, 1], F32)
        nc.sync.dma_start(out=bout_sb[:], in_=bout_d[:, :])

        idx_sb = wpool.tile([128, nslots * 8], I16)
        nc.sync.dma_start(out=idx_sb[:], in_=idx_d[:, :])
        keym_sb = wpool.tile([128, nslots], F32)
        nc.sync.dma_start(out=keym_sb[:], in_=keym_d[:, :])

        iota_sb = wpool.tile([128, 512], I16)
        nc.gpsimd.iota(iota_sb[:], pattern=[[1, 512]], base=0,
                       channel_multiplier=0,
                       allow_small_or_imprecise_dtypes=True)
        ident_sb = wpool.tile([128, 128], F32)
        make_identity(nc, ident_sb[:])

        # persistent transposed activations (xa reused for layer-2 output)
        xa = wpool.tile([128, npad], F32)   # x1T, later x3T
        xb = wpool.tile([128, npad], F32)   # x2T
        if npad > npc:
            nc.vector.memset(xa[:, npc:npad], 0.0)
            nc.vector.memset(xb[:, npc:npad], 0.0)

        # DRAM staging for the all-gather inputs
        dpool = ctx.enter_context(tc.tile_pool(name="dpool", bufs=1,
                                               space="DRAM"))
        ag1_in = dpool.tile([npc, 128], GDT)
        ag2_in = dpool.tile([npc, 128], GDT)

        # ---- helpers ----------------------------------------------------
        def leaky_inplace(ap):
            nc.vector.scalar_tensor_tensor(out=ap, in0=ap, scalar=SLOPE,
                                           in1=ap, op0=mybir.AluOpType.mult,
                                           op1=mybir.AluOpType.max)

        def transpose_to_nat(src_slice, w, nat_pool, tp_pool, ag_in):
            # src_slice: [128 feat, 128 nodes] slice of an xT tile
            ptp = tp_pool.tile([128, 128], F32, name="ptp")
            nc.tensor.transpose(ptp[:], src_slice, ident_sb[:])
            nat = nat_pool.tile([128, 128], GDT, name="nat")
            nc.vector.tensor_copy(out=nat[:], in_=ptp[:])
            wsz = min(128, npc - w * 128)
            nc.sync.dma_start(out=ag_in[w * 128: w * 128 + wsz, :],
                              in_=nat[:wsz, :])

        quart = half // 2
        rows_sub = n_cores * quart

        def emit_subag(ag_in, xfs, j):
            # quarter-granular sub-collective j: gathers each core's local
            # nodes [j*quart, (j+1)*quart) into the contiguous table rows
            # [j*rows_sub, (j+1)*rows_sub).  A segment (2 quarters) is read
            # with a base AP overlapping only the segment's SECOND quarter;
            # the first quarter's data is safe because collectives on the
            # single CC stream complete in issue order.
            nc.gpsimd.collective_compute(
                "AllGather", mybir.AluOpType.bypass, replica_groups=rg,
                ins=[ag_in[j * quart:(j + 1) * quart, :]],
                outs=[xfs[j * rows_sub:(j + 1) * rows_sub, :]])

        # ---- stage 1: x1 = leaky(tweet @ Wt + bt); leaky(x1 @ Wi + bi) --
        with tc.tile_pool(name="s1psum", bufs=2, space="PSUM") as s1psum, \
             tc.tile_pool(name="s1psum2", bufs=2, space="PSUM") as s1psum2, \
             tc.tile_pool(name="s1buf", bufs=3) as s1buf, \
             tc.tile_pool(name="s1nat", bufs=3) as s1nat, \
             tc.tile_pool(name="s1tp", bufs=2, space="PSUM") as s1tp:
            for t in range(NT):
                c0 = t * TILE_W
                cw = min(TILE_W, npc - c0)
                ps1 = s1psum.tile([128, TILE_W], F32, name="ps1")
                for k in range(KT):
                    tw = s1buf.tile([128, TILE_W], GDT, name="tw")
                    nc.sync.dma_start(
                        out=tw[:, :cw],
                        in_=tweetT_d[k * 128:(k + 1) * 128, c0:c0 + cw])
                    nc.tensor.matmul(ps1[:, :cw],
                                     lhsT=wt_sb[:, k * 128:(k + 1) * 128],
                                     rhs=tw[:, :cw],
                                     start=(k == 0), stop=(k == KT - 1))
                x1b = s1buf.tile([128, TILE_W], F32, name="x1b")
                nc.scalar.activation(out=x1b[:, :cw], in_=ps1[:, :cw],
                                     func=mybir.ActivationFunctionType.Lrelu,
                                     bias=bt_sb[:, :1], alpha=SLOPE)
                ps2 = s1psum2.tile([128, TILE_W], F32, name="ps2")
                nc.tensor.matmul(ps2[:, :cw], lhsT=wi_sb[:], rhs=x1b[:, :cw],
                                 start=True, stop=True)
                nc.vector.tensor_scalar(out=xa[:, c0:c0 + cw],
                                        in0=ps2[:, :cw],
                                        scalar1=bi_sb[:, :1], scalar2=None,
                                        op0=mybir.AluOpType.add)
                leaky_inplace(xa[:, c0:c0 + cw])
                for wi_ in range(c0 // 128, (c0 + cw + 127) // 128):
                    transpose_to_nat(xa[:, wi_ * 128:(wi_ + 1) * 128], wi_,
                                     s1nat, s1tp, ag1_in)
                # fire sub-collective j as soon as its quarter is produced
                for j in range(3):
                    if c0 < (j + 1) * quart <= c0 + cw:
                        emit_subag(ag1_in, xf1, j)
            emit_subag(ag1_in, xf1, 3)

        # ---- RGCN layers -------------------------------------------------
        # pair after which each next-layer sub-collective can fire:
        # sub-ag j needs nat rows [j*quart, (j+1)*quart)
        subag_after = {((j + 1) * quart + 255) // 256 - 1: j for j in range(3)}

        def rgcn_layer(xin, xout, xfs, ag_next, xfs_next):
            with tc.tile_pool(name="stagp", bufs=14) as stagp, \
                 tc.tile_pool(name="mp", bufs=12) as mp, \
                 tc.tile_pool(name="aggp", bufs=4, space="PSUM") as aggp, \
                 tc.tile_pool(name="meanp", bufs=3) as meanp, \
                 tc.tile_pool(name="cip", bufs=3) as cip, \
                 tc.tile_pool(name="trp", bufs=2, space="PSUM") as trp, \
                 tc.tile_pool(name="tpp", bufs=2, space="PSUM") as tpp, \
                 tc.tile_pool(name="natp", bufs=3) as natp:
                stag_tiles = {}

                def emit_gather(ci):
                    s, lo, n = calls[ci]
                    st = stagp.tile([128, MAX_CALL * 128], GDT, name="st")
                    stag_tiles[ci] = st
                    n_i = n * 128
                    nc.gpsimd.dma_gather(
                        out_ap=st[:, :n_i].rearrange("p (c d) -> p c d",
                                                     d=128),
                        in_ap=xfs[s * seg_rows + base:(s + 1) * seg_rows, :],
                        idxs_ap=idx_sb[:, lo * 8: lo * 8 + n_i // 16],
                        num_idxs=n_i,
                        num_idxs_reg=n_i,
                        elem_size=128,
                    )

                def emit_tail(p, pagg, ci_t):
                    # pair tail, emitted one pair late (software pipeline):
                    # mean (DVE), relation transform (PE), bias (ScalarE),
                    # transpose-to-nat (PE + ScalarE), next-layer sub-ags
                    mean = meanp.tile([128, 512], F32, name="mean")
                    nc.vector.tensor_tensor(out=mean[:], in0=pagg[:],
                                            in1=ci_t[:],
                                            op=mybir.AluOpType.mult)
                    ptr = trp.tile([128, 256], F32, name="ptr")
                    for wh in range(2):
                        w = p * 2 + wh
                        if w >= nw:
                            nc.vector.memset(ptr[:, wh * 128:(wh + 1) * 128],
                                             0.0)
                            continue
                        po = ptr[:, wh * 128:(wh + 1) * 128]
                        nc.tensor.matmul(
                            po, lhsT=wr0_sb[:],
                            rhs=mean[:, wh * 256:wh * 256 + 128],
                            start=True, stop=False)
                        nc.tensor.matmul(
                            po, lhsT=wr1_sb[:],
                            rhs=mean[:, wh * 256 + 128:wh * 256 + 256],
                            start=False, stop=False)
                        nc.tensor.matmul(
                            po, lhsT=wroot_sb[:],
                            rhs=xin[:, w * 128:(w + 1) * 128],
                            start=False, stop=True)
                    psz = min(256, npad - p * 256)
                    nc.scalar.activation(
                        out=xout[:, p * 256:p * 256 + psz],
                        in_=ptr[:, :psz],
                        func=mybir.ActivationFunctionType.Identity,
                        bias=brgcn_sb[:, :1])
                    if ag_next is not None:
                        for wh in range(2):
                            w = p * 2 + wh
                            if w >= nw:
                                continue
                            transpose_to_nat(xout[:, w * 128:(w + 1) * 128],
                                             w, natp, tpp, ag_next)
                        if p in subag_after:
                            emit_subag(ag_next, xfs_next, subag_after[p])
                        if p == npairs - 1:
                            emit_subag(ag_next, xfs_next, 3)

                pending = None
                for p in range(npairs):
                    psl = pair_slots[p]
                    for ci_ in prep["sched"][p]:
                        emit_gather(ci_)
                    ci_t = cip.tile([128, 512], F32, name="ci")
                    nc.sync.dma_start(out=ci_t[:],
                                      in_=cntinv_d[:, p * 512:(p + 1) * 512])
                    pagg = aggp.tile([128, 512], F32, name="pagg")
                    nmm = len(psl)
                    for i, (sl, s) in enumerate(psl):
                        ci = prep["slot_call"][sl]
                        col = prep["slot_ccol"][sl]
                        st = stag_tiles[ci]
                        m = mp.tile([128, 512], GDT, name="m")
                        nc.vector.tensor_scalar(
                            out=m[:], in0=iota_sb[:],
                            scalar1=keym_sb[:, sl:sl + 1], scalar2=None,
                            op0=mybir.AluOpType.is_equal)
                        nc.tensor.matmul(
                            pagg[:],
                            lhsT=st[:, col * 128:(col + 1) * 128],
                            rhs=m[:],
                            start=(i == 0), stop=(i == nmm - 1))
                    if pending is not None:
                        emit_tail(*pending)
                    pending = (p, pagg, ci_t)
                emit_tail(*pending)

        rgcn_layer(xa, xb, xf1, ag2_in, xf2)
        rgcn_layer(xb, xa, xf2, None, None)

        # ---- head: leaky(x @ Wo + bo) @ Wout + bout ---------------------
        with tc.tile_pool(name="hps", bufs=2, space="PSUM") as hps, \
             tc.tile_pool(name="hps2", bufs=2, space="PSUM") as hps2, \
             tc.tile_pool(name="hbuf", bufs=3) as hbuf:
            for t in range(NT):
                c0 = t * TILE_W
                cw = min(TILE_W, npc - c0)
                psh = hps.tile([128, TILE_W], F32, name="psh")
                nc.tensor.matmul(psh[:, :cw], lhsT=wo_sb[:],
                                 rhs=xa[:, c0:c0 + cw], start=True, stop=True)
                hb = hbuf.tile([128, TILE_W], F32, name="hb")
                nc.scalar.activation(out=hb[:, :cw], in_=psh[:, :cw],
                                     func=mybir.ActivationFunctionType.Lrelu,
                                     bias=bo_sb[:, :1], alpha=SLOPE)
                pso = hps2.tile([2, TILE_W], F32, name="pso")
                nc.tensor.matmul(pso[:, :cw], lhsT=wout_sb[:],
                                 rhs=hb[:, :cw], start=True, stop=True)
                ob = hbuf.tile([2, TILE_W], F32, name="ob")
                nc.vector.tensor_scalar(out=ob[:, :cw], in0=pso[:, :cw],
                                        scalar1=bout_sb[:, :1], scalar2=None,
                                        op0=mybir.AluOpType.add)
                nc.sync.dma_start(out=outT_d[:, c0:c0 + cw], in_=ob[:, :cw])

    nc.compile()
    return nc


# ---------------------------------------------------------------------------
# Public entry point
# ---------------------------------------------------------------------------

_CACHE = {}
GATHER_DTYPE = "bfloat16"


def _get_module(N, T, prep, gather_dtype=None):
    if gather_dtype is None:
        gather_dtype = GATHER_DTYPE
    key = (N, T, prep["npc"], prep["nslots"], tuple(prep["kq"].reshape(-1)),
           gather_dtype)
    if key not in _CACHE:
        _CACHE[key] = _build_module(N, T, prep, gather_dtype=gather_dtype)
    return _CACHE[key]


def _make_in_maps(tweet, prep, Wt, bt, Wi, bi, Wrel, Wroot, brgcn, Wo, bo,
                  Wout, bout, n_cores=N_CORES, gather_dtype=None):
    import ml_dtypes
    if gather_dtype is None:
        gather_dtype = GATHER_DTYPE
    gdt = np.float32 if gather_dtype == "float32" else ml_dtypes.bfloat16
    npc = prep["npc"]
    f32 = np.float32
    shared = dict(
        Wt=np.ascontiguousarray(np.asarray(Wt, f32).astype(gdt)),
        Wi=np.ascontiguousarray(Wi, f32),
        Wr0=np.ascontiguousarray(Wrel[0], f32),
        Wr1=np.ascontiguousarray(Wrel[1], f32),
        Wroot=np.ascontiguousarray(Wroot, f32),
        Wo=np.ascontiguousarray(Wo, f32),
        Wout=np.ascontiguousarray(Wout, f32),
        bt=np.ascontiguousarray(np.reshape(bt, (-1, 1)), f32),
        bi=np.ascontiguousarray(np.reshape(bi, (-1, 1)), f32),
        brgcn=np.ascontiguousarray(np.reshape(brgcn, (-1, 1)), f32),
        bo=np.ascontiguousarray(np.reshape(bo, (-1, 1)), f32),
        bout=np.ascontiguousarray(np.reshape(bout, (-1, 1)), f32),
    )
    in_maps = []
    for c in range(n_cores):
        m = dict(shared)
        m["tweetT"] = np.ascontiguousarray(
            tweet[c * npc:(c + 1) * npc].T.astype(gdt))
        m["idx16"] = np.ascontiguousarray(prep["idx16"][c])
        m["keym"] = np.ascontiguousarray(prep["keym"][c])
        m["cntinv"] = np.ascontiguousarray(prep["cntinv"][c])
        in_maps.append(m)
    return in_maps


def kernel(tweet, edge_index, edge_type, Wt, bt, Wi, bi, Wrel, Wroot, brgcn,
           Wo, bo, Wout, bout):
    tweet = np.asarray(tweet, dtype=np.float32)
    N, T = tweet.shape
    prep = _preprocess(edge_index, edge_type, N)
    nc = _get_module(N, T, prep)
    in_maps = _make_in_maps(tweet, prep, Wt, bt, Wi, bi, Wrel, Wroot, brgcn,
                            Wo, bo, Wout, bout)
    res = bass_utils.run_bass_kernel_spmd(
        nc, in_maps, core_ids=list(range(N_CORES)))
    out = np.concatenate(
        [res.results[c]["outT"].T for c in range(N_CORES)], axis=0)
    return np.ascontiguousarray(out, dtype=np.float32)


# revision 23
# speedup vs baseline: 1.0304x; 1.0145x over previous
"""BotRGCN on 8 TRN2 NeuronCores (Bass/Tile SPMD kernel), v2.

Strategy (graph/data parallel):
  - Nodes sharded across 8 cores (12500/core); edges grouped by destination
    core and 512-wide destination window-pair; 128-dim weights replicated.
  - Activations live TRANSPOSED on-chip: [feat(128 partitions), nodes(free)].
  - Per RGCN layer: the node features are all-gathered into a 2-segment HBM
    table (two pipelined sub-collectives of 50000 rows each); x[src] rows for
    local edges are fetched with dma_gather using SIGNED int16 row offsets
    from a mid-segment base (covers 50000 rows per segment); scatter-add into
    per-window-pair PSUM accumulators via one-hot matmuls:
        pagg[feat, 512] += gathered[edges, feat].T @ onehot[edges, 512]
    where onehot[e, (win&1)*256 + rel*128 + (dst&127)] = 1 (pure 0/1; built
    with a single-op is_equal on the vector engine).
  - The per-(dst,rel) mean normalization 1/cnt is folded into the PSUM->SBUF
    eviction as a tensor_tensor multiply against a host-precomputed
    [128, 512] 1/cnt tile per pair (streamed from HBM; shared by both layers).
  - Relation transform: W_r.T @ mean + Wroot.T @ xT + bias, per 128-window.

The module is one SPMD program: the per-(pair,segment) chunk counts are
compiled as the max over cores; per-core variation lives in the gather
offsets / one-hot keys / padding (key -1 => contributes nothing).
"""

import math
from contextlib import ExitStack

import numpy as np

import concourse.bacc as bacc
import concourse.bass as bass
import concourse.mybir as mybir
import concourse.tile as tile
from concourse import bass_utils
from concourse.masks import make_identity

F32 = mybir.dt.float32
BF16 = mybir.dt.bfloat16
I16 = mybir.dt.int16
SLOPE = 0.01
N_CORES = 8
NSEG = 2          # gather-table segments (signed int16 offsets span 50000)
MAX_CALL = 8      # chunk slots per dma_gather call (<=1024 idxs)


# ---------------------------------------------------------------------------
# Host-side preprocessing
# ---------------------------------------------------------------------------

def _preprocess(edge_index, edge_type, n_nodes, n_cores=N_CORES):
    src = np.asarray(edge_index[0], dtype=np.int64)
    dst = np.asarray(edge_index[1], dtype=np.int64)
    et = np.asarray(edge_type, dtype=np.int64)
    E = src.shape[0]
    npc = n_nodes // n_cores
    assert npc * n_cores == n_nodes
    half = npc // 2                      # nodes per core per sub-collective
    seg_rows = half * n_cores            # rows per table segment
    assert seg_rows <= 65534
    base = seg_rows // 2                 # gather base row within a segment
    nw = (npc + 127) // 128
    npairs = (nw + 1) // 2

    # mean weights: 1 / count(dst, rel) -> per-core per-pair [512] tiles
    segid = dst * 2 + et
    cnt = np.bincount(segid, minlength=2 * n_nodes).astype(np.float32)
    cntinv_full = 1.0 / np.maximum(cnt, 1.0)          # [2N]
    cntinv = np.ones((n_cores, npairs, 512), dtype=np.float32)
    for c in range(n_cores):
        for p in range(npairs):
            for wh in range(2):
                w = 2 * p + wh
                lo = w * 128
                n_w = min(128, npc - lo)
                if n_w <= 0:
                    continue
                nodes = c * npc + lo + np.arange(n_w)
                for r in range(2):
                    cntinv[c, p, wh * 256 + r * 128:
                           wh * 256 + r * 128 + n_w] = \
                        cntinv_full[nodes * 2 + r]

    # table row for node (c, l): quarter q = l // quart; global row =
    # q*(n_cores*quart) + c*quart + l%quart  (quarters are contiguous
    # sub-collective outputs; segment s = q//2 spans 2 quarters)
    quart = npc // 4
    rows_sub = n_cores * quart
    sc = src // npc
    sl = src - sc * npc
    q_e = sl // quart
    grow = q_e * rows_sub + sc * quart + (sl - q_e * quart)
    seg_e = grow // seg_rows
    off_e = (grow - seg_e * seg_rows) - base          # signed int16 offset

    core = dst // npc
    dstl = dst - core * npc
    win = dstl >> 7
    pair = win >> 1
    key = ((win & 1) * 256 + et * 128 + (dstl & 127)).astype(np.float32)

    # group by (core, pair, seg); chunk counts compiled as max over cores
    gid = (core * npairs + pair) * NSEG + seg_e
    counts = np.bincount(gid, minlength=n_cores * npairs * NSEG
                         ).reshape(n_cores, npairs, NSEG)
    kq = np.ceil(counts.max(axis=0) / 128).astype(np.int64)  # [npairs, NSEG]

    # slot list (pair-major), gather-call schedule (runs of <=MAX_CALL slots
    # within one segment)
    slots = []          # (pair, seg)
    pair_slots = [[] for _ in range(npairs)]
    for p in range(npairs):
        for s in range(NSEG):
            for _ in range(kq[p, s]):
                pair_slots[p].append((len(slots), s))
                slots.append((p, s))
    nslots = len(slots)
    calls = []          # (seg, slot_lo, n_sl)
    i = 0
    while i < nslots:
        s = slots[i][1]
        j = i
        while j < nslots and j - i < MAX_CALL and slots[j][1] == s:
            j += 1
        calls.append((s, i, j - i))
        i = j
    slot_call = np.zeros(nslots, dtype=np.int64)       # slot -> call idx
    slot_ccol = np.zeros(nslots, dtype=np.int64)       # slot -> col in call
    for ci, (s, lo, n) in enumerate(calls):
        slot_call[lo:lo + n] = ci
        slot_ccol[lo:lo + n] = np.arange(n)

    # gather-call emission schedule: seg-0 calls are emitted LEAD pairs
    # ahead of their first consumer so they stream while the later
    # sub-collectives (needed by seg-1) are still in flight.
    LEAD = 5
    sched = [[] for _ in range(npairs)]
    for ci, (s, lo, n) in enumerate(calls):
        first_pair = slots[lo][0]
        step = max(0, first_pair - (LEAD if s == 0 else 0))
        sched[step].append(ci)
    for step in range(npairs):       # seg-1 (just-in-time) first, except at
        # step 0 where the seg-0 lead calls must run while the last
        # sub-collective (needed by seg-1) is still in flight
        sched[step].sort(key=lambda ci: calls[ci][0] if step == 0
                         else -calls[ci][0])

    # place edges: per (core, pair, seg) sequentially into that group's slots
    order = np.argsort(gid, kind="stable")
    gid_s = gid[order]
    starts = np.zeros(n_cores * npairs * NSEG + 1, dtype=np.int64)
    np.cumsum(counts.reshape(-1), out=starts[1:])
    pos = np.arange(E, dtype=np.int64) - starts[gid_s]

    # slot base index of group (p, s): first slot of that (p,s) run
    slot_base = np.zeros((npairs, NSEG), dtype=np.int64)
    acc = 0
    for p in range(npairs):
        for s in range(NSEG):
            slot_base[p, s] = acc
            acc += kq[p, s]

    e_core = core[order]
    e_pair = pair[order]
    e_seg = seg_e[order]
    e_slot = slot_base[e_pair, e_seg] + (pos >> 7)
    e_p = pos & 127

    gidx = np.zeros((n_cores, nslots * 128), dtype=np.int16)
    keym = np.full((n_cores, 128, nslots), -1.0, dtype=np.float32)
    gidx[e_core, e_slot * 128 + e_p] = off_e[order].astype(np.int16)
    keym[e_core, e_p, e_slot] = key[order]

    # guard: the LAST idx of each call must be >= 0 (the gather ucode trims
    # trailing negatives at runtime). Pads are 0, so only calls that end with
    # a full chunk whose last edge has a negative offset need a swap.
    for c in range(n_cores):
        for (s, lo, n) in calls:
            a, b = lo * 128, (lo + n) * 128
            if gidx[c, b - 1] < 0:
                cand = np.nonzero(gidx[c, a:b] >= 0)[0]
                assert cand.size > 0, "gather call with all-negative offsets"
                j = a + cand[-1]
                f1, f2 = j, b - 1
                gidx[c, f1], gidx[c, f2] = gidx[c, f2], gidx[c, f1]
                p1, s1 = f1 & 127, f1 >> 7
                p2, s2 = f2 & 127, f2 >> 7
                tmp = keym[c, p1, s1]
                keym[c, p1, s1] = keym[c, p2, s2]
                keym[c, p2, s2] = tmp

    # wrap indices: position i -> [i%16, i//16], replicated to 128 partitions
    idx16 = np.ascontiguousarray(
        gidx.reshape(n_cores, nslots * 8, 16).transpose(0, 2, 1))
    idx16 = np.tile(idx16, (1, 8, 1))  # [n_cores, 128, nslots*8]

    cntinv_t = np.ascontiguousarray(
        np.broadcast_to(cntinv.reshape(n_cores, 1, npairs * 512),
                        (n_cores, 128, npairs * 512)))

    return dict(
        npc=npc, nw=nw, npairs=npairs, half=half, seg_rows=seg_rows,
        base=base, kq=kq, slots=slots, calls=calls, slot_call=slot_call,
        slot_ccol=slot_ccol, pair_slots=pair_slots, nslots=nslots,
        sched=sched, idx16=idx16, keym=keym, cntinv=cntinv_t,
    )


# ---------------------------------------------------------------------------
# Device kernel builder (one SPMD module for all cores)
# ---------------------------------------------------------------------------

def _build_module(N, T, prep, n_cores=N_CORES, gather_dtype="bfloat16"):
    D = 128
    KT = T // 128
    assert KT * 128 == T
    npc = prep["npc"]
    nw = prep["nw"]
    npairs = prep["npairs"]
    half = prep["half"]
    seg_rows = prep["seg_rows"]
    base = prep["base"]
    calls = prep["calls"]
    pair_slots = prep["pair_slots"]
    nslots = prep["nslots"]
    npad = nw * 128
    TILE_W = 512
    NT = (npc + TILE_W - 1) // TILE_W
    GDT = F32 if gather_dtype == "float32" else BF16

    nc = bacc.Bacc("TRN2", target_bir_lowering=False, debug=False,
                   enable_asserts=False, num_devices=n_cores)

    # ---- I/O -------------------------------------------------------------
    tweetT_d = nc.dram_tensor("tweetT", [T, npc], GDT, kind="ExternalInput")
    idx_d = nc.dram_tensor("idx16", [128, nslots * 8], I16,
                           kind="ExternalInput")
    keym_d = nc.dram_tensor("keym", [128, nslots], F32, kind="ExternalInput")
    cntinv_d = nc.dram_tensor("cntinv", [128, npairs * 512], F32,
                              kind="ExternalInput")
    Wt_d = nc.dram_tensor("Wt", [T, D], GDT, kind="ExternalInput")
    Wi_d = nc.dram_tensor("Wi", [D, D], F32, kind="ExternalInput")
    Wr0_d = nc.dram_tensor("Wr0", [D, D], F32, kind="ExternalInput")
    Wr1_d = nc.dram_tensor("Wr1", [D, D], F32, kind="ExternalInput")
    Wroot_d = nc.dram_tensor("Wroot", [D, D], F32, kind="ExternalInput")
    Wo_d = nc.dram_tensor("Wo", [D, D], F32, kind="ExternalInput")
    Wout_d = nc.dram_tensor("Wout", [D, 2], F32, kind="ExternalInput")
    bt_d = nc.dram_tensor("bt", [D, 1], F32, kind="ExternalInput")
    bi_d = nc.dram_tensor("bi", [D, 1], F32, kind="ExternalInput")
    brgcn_d = nc.dram_tensor("brgcn", [D, 1], F32, kind="ExternalInput")
    bo_d = nc.dram_tensor("bo", [D, 1], F32, kind="ExternalInput")
    bout_d = nc.dram_tensor("bout", [2, 1], F32, kind="ExternalInput")
    outT_d = nc.dram_tensor("outT", [2, npc], F32, kind="ExternalOutput")

    # all-gather output tables (4 contiguous quarter-collectives each)
    xf1 = nc.dram_tensor("xfull1", [N, 128], GDT)
    xf2 = nc.dram_tensor("xfull2", [N, 128], GDT)

    rg = [list(range(n_cores))]

    with tile.TileContext(nc) as tc, ExitStack() as ctx:
        # ---- persistent SBUF state --------------------------------------
        wpool = ctx.enter_context(tc.tile_pool(name="wpool", bufs=1))
        wt_sb = wpool.tile([128, KT * 128], GDT)
        for k in range(KT):
            nc.sync.dma_start(out=wt_sb[:, k * 128:(k + 1) * 128],
                              in_=Wt_d[k * 128:(k + 1) * 128, :])
        wi_sb = wpool.tile([128, 128], F32)
        nc.sync.dma_start(out=wi_sb[:], in_=Wi_d[:, :])
        wr0_sb = wpool.tile([128, 128], F32)
        nc.sync.dma_start(out=wr0_sb[:], in_=Wr0_d[:, :])
        wr1_sb = wpool.tile([128, 128], F32)
        nc.sync.dma_start(out=wr1_sb[:], in_=Wr1_d[:, :])
        wroot_sb = wpool.tile([128, 128], F32)
        nc.sync.dma_start(out=wroot_sb[:], in_=Wroot_d[:, :])
        wo_sb = wpool.tile([128, 128], F32)
        nc.sync.dma_start(out=wo_sb[:], in_=Wo_d[:, :])
        wout_sb = wpool.tile([128, 2], F32)
        nc.sync.dma_start(out=wout_sb[:], in_=Wout_d[:, :])
        bt_sb = wpool.tile([128, 1], F32)
        nc.sync.dma_start(out=bt_sb[:], in_=bt_d[:, :])
        bi_sb = wpool.tile([128, 1], F32)
        nc.sync.dma_start(out=bi_sb[:], in_=bi_d[:, :])
        brgcn_sb = wpool.tile([128, 1], F32)
        nc.sync.dma_start(out=brgcn_sb[:], in_=brgcn_d[:, :])
        bo_sb = wpool.tile([128, 1], F32)
        nc.sync.dma_start(out=bo_sb[:], in_=bo_d[:, :])
        bout_sb = wpool.tile([2, 1], F32)
        nc.sync.dma_start(out=bout_sb[:], in_=bout_d[:, :])

        idx_sb = wpool.tile([128, nslots * 8], I16)
        nc.sync.dma_start(out=idx_sb[:], in_=idx_d[:, :])
        keym_sb = wpool.tile([128, nslots], F32)
        nc.sync.dma_start(out=keym_sb[:], in_=keym_d[:, :])

        iota_sb = wpool.tile([128, 512], I16)
        nc.gpsimd.iota(iota_sb[:], pattern=[[1, 512]], base=0,
                       channel_multiplier=0,
                       allow_small_or_imprecise_dtypes=True)
        ident_sb = wpool.tile([128, 128], F32)
        make_identity(nc, ident_sb[:])

        # persistent transposed activations (xa reused for layer-2 output)
        xa = wpool.tile([128, npad], F32)   # x1T, later x3T
        xb = wpool.tile([128, npad], F32)   # x2T
        if npad > npc:
            nc.vector.memset(xa[:, npc:npad], 0.0)
            nc.vector.memset(xb[:, npc:npad], 0.0)

        # DRAM staging for the all-gather inputs
        dpool = ctx.enter_context(tc.tile_pool(name="dpool", bufs=1,
                                               space="DRAM"))
        ag1_in = dpool.tile([npc, 128], GDT)
        ag2_in = dpool.tile([npc, 128], GDT)

        # ---- helpers ----------------------------------------------------
        def leaky_inplace(ap):
            nc.vector.scalar_tensor_tensor(out=ap, in0=ap, scalar=SLOPE,
                                           in1=ap, op0=mybir.AluOpType.mult,
                                           op1=mybir.AluOpType.max)

        def transpose_to_nat(src_slice, w, nat_pool, tp_pool, ag_in):
            # src_slice: [128 feat, 128 nodes] slice of an xT tile
            ptp = tp_pool.tile([128, 128], F32, name="ptp")
            nc.tensor.transpose(ptp[:], src_slice, ident_sb[:])
            nat = nat_pool.tile([128, 128], GDT, name="nat")
            nc.vector.tensor_copy(out=nat[:], in_=ptp[:])
            wsz = min(128, npc - w * 128)
            nc.sync.dma_start(out=ag_in[w * 128: w * 128 + wsz, :],
                              in_=nat[:wsz, :])

        quart = half // 2
        rows_sub = n_cores * quart

        def emit_subag(ag_in, xfs, j):
            # quarter-granular sub-collective j: gathers each core's local
            # nodes [j*quart, (j+1)*quart) into the contiguous table rows
            # [j*rows_sub, (j+1)*rows_sub).  A segment (2 quarters) is read
            # with a base AP overlapping only the segment's SECOND quarter;
            # the first quarter's data is safe because collectives on the
            # single CC stream complete in issue order.
            nc.gpsimd.collective_compute(
                "AllGather", mybir.AluOpType.bypass, replica_groups=rg,
                ins=[ag_in[j * quart:(j + 1) * quart, :]],
                outs=[xfs[j * rows_sub:(j + 1) * rows_sub, :]])

        # ---- stage 1: x1 = leaky(tweet @ Wt + bt); leaky(x1 @ Wi + bi) --
        with tc.tile_pool(name="s1psum", bufs=2, space="PSUM") as s1psum, \
             tc.tile_pool(name="s1psum2", bufs=2, space="PSUM") as s1psum2, \
             tc.tile_pool(name="s1buf", bufs=8) as s1buf, \
             tc.tile_pool(name="s1nat", bufs=3) as s1nat, \
             tc.tile_pool(name="s1tp", bufs=2, space="PSUM") as s1tp:
            for t in range(NT):
                c0 = t * TILE_W
                cw = min(TILE_W, npc - c0)
                ps1 = s1psum.tile([128, TILE_W], F32, name="ps1")
                for k in range(KT):
                    tw = s1buf.tile([128, TILE_W], GDT, name="tw")
                    nc.sync.dma_start(
                        out=tw[:, :cw],
                        in_=tweetT_d[k * 128:(k + 1) * 128, c0:c0 + cw])
                    nc.tensor.matmul(ps1[:, :cw],
                                     lhsT=wt_sb[:, k * 128:(k + 1) * 128],
                                     rhs=tw[:, :cw],
                                     start=(k == 0), stop=(k == KT - 1))
                x1b = s1buf.tile([128, TILE_W], F32, name="x1b")
                nc.scalar.activation(out=x1b[:, :cw], in_=ps1[:, :cw],
                                     func=mybir.ActivationFunctionType.Lrelu,
                                     bias=bt_sb[:, :1], alpha=SLOPE)
                ps2 = s1psum2.tile([128, TILE_W], F32, name="ps2")
                nc.tensor.matmul(ps2[:, :cw], lhsT=wi_sb[:], rhs=x1b[:, :cw],
                                 start=True, stop=True)
                nc.vector.tensor_scalar(out=xa[:, c0:c0 + cw],
                                        in0=ps2[:, :cw],
                                        scalar1=bi_sb[:, :1], scalar2=None,
                                        op0=mybir.AluOpType.add)
                leaky_inplace(xa[:, c0:c0 + cw])
                for wi_ in range(c0 // 128, (c0 + cw + 127) // 128):
                    transpose_to_nat(xa[:, wi_ * 128:(wi_ + 1) * 128], wi_,
                                     s1nat, s1tp, ag1_in)
                # fire sub-collective j as soon as its quarter is produced
                for j in range(3):
                    if c0 < (j + 1) * quart <= c0 + cw:
                        emit_subag(ag1_in, xf1, j)
            emit_subag(ag1_in, xf1, 3)

        # ---- RGCN layers -------------------------------------------------
        # pair after which each next-layer sub-collective can fire:
        # sub-ag j needs nat rows [j*quart, (j+1)*quart)
        subag_after = {((j + 1) * quart + 255) // 256 - 1: j for j in range(3)}

        def rgcn_layer(xin, xout, xfs, ag_next, xfs_next):
            with tc.tile_pool(name="stagp", bufs=14) as stagp, \
                 tc.tile_pool(name="mp", bufs=12) as mp, \
                 tc.tile_pool(name="aggp", bufs=4, space="PSUM") as aggp, \
                 tc.tile_pool(name="meanp", bufs=3) as meanp, \
                 tc.tile_pool(name="cip", bufs=3) as cip, \
                 tc.tile_pool(name="trp", bufs=2, space="PSUM") as trp, \
                 tc.tile_pool(name="tpp", bufs=2, space="PSUM") as tpp, \
                 tc.tile_pool(name="natp", bufs=3) as natp:
                stag_tiles = {}

                def emit_gather(ci):
                    s, lo, n = calls[ci]
                    st = stagp.tile([128, MAX_CALL * 128], GDT, name="st")
                    stag_tiles[ci] = st
                    n_i = n * 128
                    nc.gpsimd.dma_gather(
                        out_ap=st[:, :n_i].rearrange("p (c d) -> p c d",
                                                     d=128),
                        in_ap=xfs[s * seg_rows + base:(s + 1) * seg_rows, :],
                        idxs_ap=idx_sb[:, lo * 8: lo * 8 + n_i // 16],
                        num_idxs=n_i,
                        num_idxs_reg=n_i,
                        elem_size=128,
                    )

                def emit_tail(p, pagg, ci_t):
                    # pair tail, emitted one pair late (software pipeline):
                    # mean (DVE), relation transform (PE), bias (ScalarE),
                    # transpose-to-nat (PE + ScalarE), next-layer sub-ags
                    mean = meanp.tile([128, 512], F32, name="mean")
                    nc.vector.tensor_tensor(out=mean[:], in0=pagg[:],
                                            in1=ci_t[:],
                                            op=mybir.AluOpType.mult)
                    ptr = trp.tile([128, 256], F32, name="ptr")
                    for wh in range(2):
                        w = p * 2 + wh
                        if w >= nw:
                            nc.vector.memset(ptr[:, wh * 128:(wh + 1) * 128],
                                             0.0)
                            continue
                        po = ptr[:, wh * 128:(wh + 1) * 128]
                        nc.tensor.matmul(
                            po, lhsT=wr0_sb[:],
                            rhs=mean[:, wh * 256:wh * 256 + 128],
                            start=True, stop=False)
                        nc.tensor.matmul(
                            po, lhsT=wr1_sb[:],
                            rhs=mean[:, wh * 256 + 128:wh * 256 + 256],
                            start=False, stop=False)
                        nc.tensor.matmul(
                            po, lhsT=wroot_sb[:],
                            rhs=xin[:, w * 128:(w + 1) * 128],
                            start=False, stop=True)
                    psz = min(256, npad - p * 256)
                    nc.scalar.activation(
                        out=xout[:, p * 256:p * 256 + psz],
                        in_=ptr[:, :psz],
                        func=mybir.ActivationFunctionType.Identity,
                        bias=brgcn_sb[:, :1])
                    if ag_next is not None:
                        for wh in range(2):
                            w = p * 2 + wh
                            if w >= nw:
                                continue
                            transpose_to_nat(xout[:, w * 128:(w + 1) * 128],
                                             w, natp, tpp, ag_next)
                        if p in subag_after:
                            emit_subag(ag_next, xfs_next, subag_after[p])
                        if p == npairs - 1:
                            emit_subag(ag_next, xfs_next, 3)

                pending = None
                for p in range(npairs):
                    psl = pair_slots[p]
                    for ci_ in prep["sched"][p]:
                        emit_gather(ci_)
                    ci_t = cip.tile([128, 512], F32, name="ci")
                    nc.sync.dma_start(out=ci_t[:],
                                      in_=cntinv_d[:, p * 512:(p + 1) * 512])
                    pagg = aggp.tile([128, 512], F32, name="pagg")
                    nmm = len(psl)
                    for i, (sl, s) in enumerate(psl):
                        ci = prep["slot_call"][sl]
                        col = prep["slot_ccol"][sl]
                        st = stag_tiles[ci]
                        m = mp.tile([128, 512], GDT, name="m")
                        nc.vector.tensor_scalar(
                            out=m[:], in0=iota_sb[:],
                            scalar1=keym_sb[:, sl:sl + 1], scalar2=None,
                            op0=mybir.AluOpType.is_equal)
                        nc.tensor.matmul(
                            pagg[:],
                            lhsT=st[:, col * 128:(col + 1) * 128],
                            rhs=m[:],
                            start=(i == 0), stop=(i == nmm - 1))
                    if pending is not None:
                        emit_tail(*pending)
                    pending = (p, pagg, ci_t)
                emit_tail(*pending)

        rgcn_layer(xa, xb, xf1, ag2_in, xf2)
        rgcn_layer(xb, xa, xf2, None, None)

        # ---- head: leaky(x @ Wo + bo) @ Wout + bout ---------------------
        with tc.tile_pool(name="hps", bufs=2, space="PSUM") as hps, \
             tc.tile_pool(name="hps2", bufs=2, space="PSUM") as hps2, \
             tc.tile_pool(name="hbuf", bufs=3) as hbuf:
            for t in range(NT):
                c0 = t * TILE_W
                cw = min(TILE_W, npc - c0)
                psh = hps.tile([128, TILE_W], F32, name="psh")
                nc.tensor.matmul(psh[:, :cw], lhsT=wo_sb[:],
                                 rhs=xa[:, c0:c0 + cw], start=True, stop=True)
                hb = hbuf.tile([128, TILE_W], F32, name="hb")
                nc.scalar.activation(out=hb[:, :cw], in_=psh[:, :cw],
                                     func=mybir.ActivationFunctionType.Lrelu,
                                     bias=bo_sb[:, :1], alpha=SLOPE)
                pso = hps2.tile([2, TILE_W], F32, name="pso")
                nc.tensor.matmul(pso[:, :cw], lhsT=wout_sb[:],
                                 rhs=hb[:, :cw], start=True, stop=True)
                ob = hbuf.tile([2, TILE_W], F32, name="ob")
                nc.vector.tensor_scalar(out=ob[:, :cw], in0=pso[:, :cw],
                                        scalar1=bout_sb[:, :1], scalar2=None,
                                        op0=mybir.AluOpType.add)
                nc.sync.dma_start(out=outT_d[:, c0:c0 + cw], in_=ob[:, :cw])

    nc.compile()
    return nc


# ---------------------------------------------------------------------------
# Public entry point
# ---------------------------------------------------------------------------

_CACHE = {}
GATHER_DTYPE = "bfloat16"


def _get_module(N, T, prep, gather_dtype=None):
    if gather_dtype is None:
        gather_dtype = GATHER_DTYPE
    key = (N, T, prep["npc"], prep["nslots"], tuple(prep["kq"].reshape(-1)),
           gather_dtype)
    if key not in _CACHE:
        _CACHE[key] = _build_module(N, T, prep, gather_dtype=gather_dtype)
    return _CACHE[key]


def _make_in_maps(tweet, prep, Wt, bt, Wi, bi, Wrel, Wroot, brgcn, Wo, bo,
                  Wout, bout, n_cores=N_CORES, gather_dtype=None):
    import ml_dtypes
    if gather_dtype is None:
        gather_dtype = GATHER_DTYPE
    gdt = np.float32 if gather_dtype == "float32" else ml_dtypes.bfloat16
    npc = prep["npc"]
    f32 = np.float32
    shared = dict(
        Wt=np.ascontiguousarray(np.asarray(Wt, f32).astype(gdt)),
        Wi=np.ascontiguousarray(Wi, f32),
        Wr0=np.ascontiguousarray(Wrel[0], f32),
        Wr1=np.ascontiguousarray(Wrel[1], f32),
        Wroot=np.ascontiguousarray(Wroot, f32),
        Wo=np.ascontiguousarray(Wo, f32),
        Wout=np.ascontiguousarray(Wout, f32),
        bt=np.ascontiguousarray(np.reshape(bt, (-1, 1)), f32),
        bi=np.ascontiguousarray(np.reshape(bi, (-1, 1)), f32),
        brgcn=np.ascontiguousarray(np.reshape(brgcn, (-1, 1)), f32),
        bo=np.ascontiguousarray(np.reshape(bo, (-1, 1)), f32),
        bout=np.ascontiguousarray(np.reshape(bout, (-1, 1)), f32),
    )
    in_maps = []
    for c in range(n_cores):
        m = dict(shared)
        m["tweetT"] = np.ascontiguousarray(
            tweet[c * npc:(c + 1) * npc].T.astype(gdt))
        m["idx16"] = np.ascontiguousarray(prep["idx16"][c])
        m["keym"] = np.ascontiguousarray(prep["keym"][c])
        m["cntinv"] = np.ascontiguousarray(prep["cntinv"][c])
        in_maps.append(m)
    return in_maps


def kernel(tweet, edge_index, edge_type, Wt, bt, Wi, bi, Wrel, Wroot, brgcn,
           Wo, bo, Wout, bout):
    tweet = np.asarray(tweet, dtype=np.float32)
    N, T = tweet.shape
    prep = _preprocess(edge_index, edge_type, N)
    nc = _get_module(N, T, prep)
    in_maps = _make_in_maps(tweet, prep, Wt, bt, Wi, bi, Wrel, Wroot, brgcn,
                            Wo, bo, Wout, bout)
    res = bass_utils.run_bass_kernel_spmd(
        nc, in_maps, core_ids=list(range(N_CORES)))
    out = np.concatenate(
        [res.results[c]["outT"].T for c in range(N_CORES)], axis=0)
    return np.ascontiguousarray(out, dtype=np.float32)


# revision 24
# speedup vs baseline: 1.0662x; 1.0347x over previous
"""BotRGCN on 8 TRN2 NeuronCores (Bass/Tile SPMD kernel), v2.

Strategy (graph/data parallel):
  - Nodes sharded across 8 cores (12500/core); edges grouped by destination
    core and 512-wide destination window-pair; 128-dim weights replicated.
  - Activations live TRANSPOSED on-chip: [feat(128 partitions), nodes(free)].
  - Per RGCN layer: the node features are all-gathered into a 2-segment HBM
    table (two pipelined sub-collectives of 50000 rows each); x[src] rows for
    local edges are fetched with dma_gather using SIGNED int16 row offsets
    from a mid-segment base (covers 50000 rows per segment); scatter-add into
    per-window-pair PSUM accumulators via one-hot matmuls:
        pagg[feat, 512] += gathered[edges, feat].T @ onehot[edges, 512]
    where onehot[e, (win&1)*256 + rel*128 + (dst&127)] = 1 (pure 0/1; built
    with a single-op is_equal on the vector engine).
  - The per-(dst,rel) mean normalization 1/cnt is folded into the PSUM->SBUF
    eviction as a tensor_tensor multiply against a host-precomputed
    [128, 512] 1/cnt tile per pair (streamed from HBM; shared by both layers).
  - Relation transform: W_r.T @ mean + Wroot.T @ xT + bias, per 128-window.

The module is one SPMD program: the per-(pair,segment) chunk counts are
compiled as the max over cores; per-core variation lives in the gather
offsets / one-hot keys / padding (key -1 => contributes nothing).
"""

import math
from contextlib import ExitStack

import numpy as np

import concourse.bacc as bacc
import concourse.bass as bass
import concourse.mybir as mybir
import concourse.tile as tile
from concourse import bass_utils
from concourse.masks import make_identity

F32 = mybir.dt.float32
BF16 = mybir.dt.bfloat16
I16 = mybir.dt.int16
SLOPE = 0.01
N_CORES = 8
NSEG = 2          # gather-table segments (signed int16 offsets span 50000)
MAX_CALL = 8      # chunk slots per dma_gather call (<=1024 idxs)


# ---------------------------------------------------------------------------
# Host-side preprocessing
# ---------------------------------------------------------------------------

def _preprocess(edge_index, edge_type, n_nodes, n_cores=N_CORES):
    src = np.asarray(edge_index[0], dtype=np.int64)
    dst = np.asarray(edge_index[1], dtype=np.int64)
    et = np.asarray(edge_type, dtype=np.int64)
    E = src.shape[0]
    npc = n_nodes // n_cores
    assert npc * n_cores == n_nodes
    half = npc // 2                      # nodes per core per sub-collective
    seg_rows = half * n_cores            # rows per table segment
    assert seg_rows <= 65534
    base = seg_rows // 2                 # gather base row within a segment
    nw = (npc + 127) // 128
    npairs = (nw + 1) // 2

    # mean weights: 1 / count(dst, rel) -> per-core per-pair [512] tiles
    segid = dst * 2 + et
    cnt = np.bincount(segid, minlength=2 * n_nodes).astype(np.float32)
    cntinv_full = 1.0 / np.maximum(cnt, 1.0)          # [2N]
    cntinv = np.ones((n_cores, npairs, 512), dtype=np.float32)
    for c in range(n_cores):
        for p in range(npairs):
            for wh in range(2):
                w = 2 * p + wh
                lo = w * 128
                n_w = min(128, npc - lo)
                if n_w <= 0:
                    continue
                nodes = c * npc + lo + np.arange(n_w)
                for r in range(2):
                    cntinv[c, p, wh * 256 + r * 128:
                           wh * 256 + r * 128 + n_w] = \
                        cntinv_full[nodes * 2 + r]

    # table row for node (c, l): quarter q = l // quart; global row =
    # q*(n_cores*quart) + c*quart + l%quart  (quarters are contiguous
    # sub-collective outputs; segment s = q//2 spans 2 quarters)
    quart = npc // 4
    rows_sub = n_cores * quart
    sc = src // npc
    sl = src - sc * npc
    q_e = sl // quart
    grow = q_e * rows_sub + sc * quart + (sl - q_e * quart)
    seg_e = grow // seg_rows
    off_e = (grow - seg_e * seg_rows) - base          # signed int16 offset

    core = dst // npc
    dstl = dst - core * npc
    win = dstl >> 7
    pair = win >> 1
    key = ((win & 1) * 256 + et * 128 + (dstl & 127)).astype(np.float32)

    # group by (core, pair, seg); chunk counts compiled as max over cores
    gid = (core * npairs + pair) * NSEG + seg_e
    counts = np.bincount(gid, minlength=n_cores * npairs * NSEG
                         ).reshape(n_cores, npairs, NSEG)
    kq = np.ceil(counts.max(axis=0) / 128).astype(np.int64)  # [npairs, NSEG]

    # slot list (pair-major), gather-call schedule (runs of <=MAX_CALL slots
    # within one segment)
    slots = []          # (pair, seg)
    pair_slots = [[] for _ in range(npairs)]
    for p in range(npairs):
        for s in range(NSEG):
            for _ in range(kq[p, s]):
                pair_slots[p].append((len(slots), s))
                slots.append((p, s))
    nslots = len(slots)
    calls = []          # (seg, slot_lo, n_sl)
    i = 0
    while i < nslots:
        s = slots[i][1]
        j = i
        while j < nslots and j - i < MAX_CALL and slots[j][1] == s:
            j += 1
        calls.append((s, i, j - i))
        i = j
    slot_call = np.zeros(nslots, dtype=np.int64)       # slot -> call idx
    slot_ccol = np.zeros(nslots, dtype=np.int64)       # slot -> col in call
    for ci, (s, lo, n) in enumerate(calls):
        slot_call[lo:lo + n] = ci
        slot_ccol[lo:lo + n] = np.arange(n)

    # gather-call emission schedule: seg-0 calls are emitted LEAD pairs
    # ahead of their first consumer so they stream while the later
    # sub-collectives (needed by seg-1) are still in flight.
    LEAD = 5
    sched = [[] for _ in range(npairs)]
    for ci, (s, lo, n) in enumerate(calls):
        first_pair = slots[lo][0]
        step = max(0, first_pair - (LEAD if s == 0 else 0))
        sched[step].append(ci)
    for step in range(npairs):       # seg-1 (just-in-time) first, except at
        # step 0 where the seg-0 lead calls must run while the last
        # sub-collective (needed by seg-1) is still in flight
        sched[step].sort(key=lambda ci: calls[ci][0] if step == 0
                         else -calls[ci][0])

    # place edges: per (core, pair, seg) sequentially into that group's slots
    order = np.argsort(gid, kind="stable")
    gid_s = gid[order]
    starts = np.zeros(n_cores * npairs * NSEG + 1, dtype=np.int64)
    np.cumsum(counts.reshape(-1), out=starts[1:])
    pos = np.arange(E, dtype=np.int64) - starts[gid_s]

    # slot base index of group (p, s): first slot of that (p,s) run
    slot_base = np.zeros((npairs, NSEG), dtype=np.int64)
    acc = 0
    for p in range(npairs):
        for s in range(NSEG):
            slot_base[p, s] = acc
            acc += kq[p, s]

    e_core = core[order]
    e_pair = pair[order]
    e_seg = seg_e[order]
    e_slot = slot_base[e_pair, e_seg] + (pos >> 7)
    e_p = pos & 127

    gidx = np.zeros((n_cores, nslots * 128), dtype=np.int16)
    keym = np.full((n_cores, 128, nslots), -1.0, dtype=np.float32)
    gidx[e_core, e_slot * 128 + e_p] = off_e[order].astype(np.int16)
    keym[e_core, e_p, e_slot] = key[order]

    # guard: the LAST idx of each call must be >= 0 (the gather ucode trims
    # trailing negatives at runtime). Pads are 0, so only calls that end with
    # a full chunk whose last edge has a negative offset need a swap.
    for c in range(n_cores):
        for (s, lo, n) in calls:
            a, b = lo * 128, (lo + n) * 128
            if gidx[c, b - 1] < 0:
                cand = np.nonzero(gidx[c, a:b] >= 0)[0]
                assert cand.size > 0, "gather call with all-negative offsets"
                j = a + cand[-1]
                f1, f2 = j, b - 1
                gidx[c, f1], gidx[c, f2] = gidx[c, f2], gidx[c, f1]
                p1, s1 = f1 & 127, f1 >> 7
                p2, s2 = f2 & 127, f2 >> 7
                tmp = keym[c, p1, s1]
                keym[c, p1, s1] = keym[c, p2, s2]
                keym[c, p2, s2] = tmp

    # wrap indices: position i -> [i%16, i//16], replicated to 128 partitions
    idx16 = np.ascontiguousarray(
        gidx.reshape(n_cores, nslots * 8, 16).transpose(0, 2, 1))
    idx16 = np.tile(idx16, (1, 8, 1))  # [n_cores, 128, nslots*8]

    cntinv_t = np.ascontiguousarray(
        np.broadcast_to(cntinv.reshape(n_cores, 1, npairs * 512),
                        (n_cores, 128, npairs * 512)))

    return dict(
        npc=npc, nw=nw, npairs=npairs, half=half, seg_rows=seg_rows,
        base=base, kq=kq, slots=slots, calls=calls, slot_call=slot_call,
        slot_ccol=slot_ccol, pair_slots=pair_slots, nslots=nslots,
        sched=sched, idx16=idx16, keym=keym, cntinv=cntinv_t,
    )


# ---------------------------------------------------------------------------
# Device kernel builder (one SPMD module for all cores)
# ---------------------------------------------------------------------------

def _build_module(N, T, prep, n_cores=N_CORES, gather_dtype="bfloat16"):
    D = 128
    KT = T // 128
    assert KT * 128 == T
    npc = prep["npc"]
    nw = prep["nw"]
    npairs = prep["npairs"]
    half = prep["half"]
    seg_rows = prep["seg_rows"]
    base = prep["base"]
    calls = prep["calls"]
    pair_slots = prep["pair_slots"]
    nslots = prep["nslots"]
    npad = nw * 128
    TILE_W = 512
    NT = (npc + TILE_W - 1) // TILE_W
    GDT = F32 if gather_dtype == "float32" else BF16

    nc = bacc.Bacc("TRN2", target_bir_lowering=False, debug=False,
                   enable_asserts=False, num_devices=n_cores)

    # ---- I/O -------------------------------------------------------------
    tweetT_d = nc.dram_tensor("tweetT", [T, npc], GDT, kind="ExternalInput")
    idx_d = nc.dram_tensor("idx16", [128, nslots * 8], I16,
                           kind="ExternalInput")
    keym_d = nc.dram_tensor("keym", [128, nslots], F32, kind="ExternalInput")
    cntinv_d = nc.dram_tensor("cntinv", [128, npairs * 512], F32,
                              kind="ExternalInput")
    Wt_d = nc.dram_tensor("Wt", [T, D], GDT, kind="ExternalInput")
    Wi_d = nc.dram_tensor("Wi", [D, D], F32, kind="ExternalInput")
    Wr0_d = nc.dram_tensor("Wr0", [D, D], F32, kind="ExternalInput")
    Wr1_d = nc.dram_tensor("Wr1", [D, D], F32, kind="ExternalInput")
    Wroot_d = nc.dram_tensor("Wroot", [D, D], F32, kind="ExternalInput")
    Wo_d = nc.dram_tensor("Wo", [D, D], F32, kind="ExternalInput")
    Wout_d = nc.dram_tensor("Wout", [D, 2], F32, kind="ExternalInput")
    bt_d = nc.dram_tensor("bt", [D, 1], F32, kind="ExternalInput")
    bi_d = nc.dram_tensor("bi", [D, 1], F32, kind="ExternalInput")
    brgcn_d = nc.dram_tensor("brgcn", [D, 1], F32, kind="ExternalInput")
    bo_d = nc.dram_tensor("bo", [D, 1], F32, kind="ExternalInput")
    bout_d = nc.dram_tensor("bout", [2, 1], F32, kind="ExternalInput")
    outT_d = nc.dram_tensor("outT", [2, npc], F32, kind="ExternalOutput")

    # all-gather output tables (4 contiguous quarter-collectives each)
    xf1 = nc.dram_tensor("xfull1", [N, 128], GDT)
    xf2 = nc.dram_tensor("xfull2", [N, 128], GDT)

    rg = [list(range(n_cores))]

    with tile.TileContext(nc) as tc, ExitStack() as ctx:
        # ---- persistent SBUF state --------------------------------------
        wpool = ctx.enter_context(tc.tile_pool(name="wpool", bufs=1))
        wt_sb = wpool.tile([128, KT * 128], GDT)
        for k in range(KT):
            nc.sync.dma_start(out=wt_sb[:, k * 128:(k + 1) * 128],
                              in_=Wt_d[k * 128:(k + 1) * 128, :])
        wi_sb = wpool.tile([128, 128], F32)
        nc.sync.dma_start(out=wi_sb[:], in_=Wi_d[:, :])
        wr0_sb = wpool.tile([128, 128], F32)
        nc.sync.dma_start(out=wr0_sb[:], in_=Wr0_d[:, :])
        wr1_sb = wpool.tile([128, 128], F32)
        nc.sync.dma_start(out=wr1_sb[:], in_=Wr1_d[:, :])
        wroot_sb = wpool.tile([128, 128], F32)
        nc.sync.dma_start(out=wroot_sb[:], in_=Wroot_d[:, :])
        wo_sb = wpool.tile([128, 128], F32)
        nc.sync.dma_start(out=wo_sb[:], in_=Wo_d[:, :])
        wout_sb = wpool.tile([128, 2], F32)
        nc.sync.dma_start(out=wout_sb[:], in_=Wout_d[:, :])
        bt_sb = wpool.tile([128, 1], F32)
        nc.sync.dma_start(out=bt_sb[:], in_=bt_d[:, :])
        bi_sb = wpool.tile([128, 1], F32)
        nc.sync.dma_start(out=bi_sb[:], in_=bi_d[:, :])
        brgcn_sb = wpool.tile([128, 1], F32)
        nc.sync.dma_start(out=brgcn_sb[:], in_=brgcn_d[:, :])
        bo_sb = wpool.tile([128, 1], F32)
        nc.sync.dma_start(out=bo_sb[:], in_=bo_d[:, :])
        bout_sb = wpool.tile([2, 1], F32)
        nc.sync.dma_start(out=bout_sb[:], in_=bout_d[:, :])

        idx_sb = wpool.tile([128, nslots * 8], I16)
        nc.sync.dma_start(out=idx_sb[:], in_=idx_d[:, :])
        keym_sb = wpool.tile([128, nslots], F32)
        nc.sync.dma_start(out=keym_sb[:], in_=keym_d[:, :])

        iota_sb = wpool.tile([128, 512], I16)
        nc.gpsimd.iota(iota_sb[:], pattern=[[1, 512]], base=0,
                       channel_multiplier=0,
                       allow_small_or_imprecise_dtypes=True)
        ident_sb = wpool.tile([128, 128], F32)
        make_identity(nc, ident_sb[:])

        # persistent transposed activations (xa reused for layer-2 output)
        xa = wpool.tile([128, npad], F32)   # x1T, later x3T
        xb = wpool.tile([128, npad], F32)   # x2T
        if npad > npc:
            nc.vector.memset(xa[:, npc:npad], 0.0)
            nc.vector.memset(xb[:, npc:npad], 0.0)

        # DRAM staging for the all-gather inputs
        dpool = ctx.enter_context(tc.tile_pool(name="dpool", bufs=1,
                                               space="DRAM"))
        ag1_in = dpool.tile([npc, 128], GDT)
        ag2_in = dpool.tile([npc, 128], GDT)

        # ---- helpers ----------------------------------------------------
        def leaky_inplace(ap):
            nc.vector.scalar_tensor_tensor(out=ap, in0=ap, scalar=SLOPE,
                                           in1=ap, op0=mybir.AluOpType.mult,
                                           op1=mybir.AluOpType.max)

        def transpose_to_nat(src_slice, w, nat_pool, tp_pool, ag_in):
            # src_slice: [128 feat, 128 nodes] slice of an xT tile
            ptp = tp_pool.tile([128, 128], F32, name="ptp")
            nc.tensor.transpose(ptp[:], src_slice, ident_sb[:])
            nat = nat_pool.tile([128, 128], GDT, name="nat")
            nc.vector.tensor_copy(out=nat[:], in_=ptp[:])
            wsz = min(128, npc - w * 128)
            nc.sync.dma_start(out=ag_in[w * 128: w * 128 + wsz, :],
                              in_=nat[:wsz, :])

        quart = half // 2
        rows_sub = n_cores * quart

        def emit_subag(ag_in, xfs, j):
            # quarter-granular sub-collective j: gathers each core's local
            # nodes [j*quart, (j+1)*quart) into the contiguous table rows
            # [j*rows_sub, (j+1)*rows_sub).  A segment (2 quarters) is read
            # with a base AP overlapping only the segment's SECOND quarter;
            # the first quarter's data is safe because collectives on the
            # single CC stream complete in issue order.
            nc.gpsimd.collective_compute(
                "AllGather", mybir.AluOpType.bypass, replica_groups=rg,
                ins=[ag_in[j * quart:(j + 1) * quart, :]],
                outs=[xfs[j * rows_sub:(j + 1) * rows_sub, :]])

        # ---- stage 1: x1 = leaky(tweet @ Wt + bt); leaky(x1 @ Wi + bi) --
        with tc.tile_pool(name="s1psum", bufs=2, space="PSUM") as s1psum, \
             tc.tile_pool(name="s1psum2", bufs=2, space="PSUM") as s1psum2, \
             tc.tile_pool(name="s1buf", bufs=8) as s1buf, \
             tc.tile_pool(name="s1nat", bufs=3) as s1nat, \
             tc.tile_pool(name="s1tp", bufs=2, space="PSUM") as s1tp:
            for t in range(NT):
                c0 = t * TILE_W
                cw = min(TILE_W, npc - c0)
                ps1 = s1psum.tile([128, TILE_W], F32, name="ps1")
                for k in range(KT):
                    tw = s1buf.tile([128, TILE_W], GDT, name="tw")
                    nc.sync.dma_start(
                        out=tw[:, :cw],
                        in_=tweetT_d[k * 128:(k + 1) * 128, c0:c0 + cw])
                    nc.tensor.matmul(ps1[:, :cw],
                                     lhsT=wt_sb[:, k * 128:(k + 1) * 128],
                                     rhs=tw[:, :cw],
                                     start=(k == 0), stop=(k == KT - 1))
                x1b = s1buf.tile([128, TILE_W], F32, name="x1b")
                nc.scalar.activation(out=x1b[:, :cw], in_=ps1[:, :cw],
                                     func=mybir.ActivationFunctionType.Lrelu,
                                     bias=bt_sb[:, :1], alpha=SLOPE)
                ps2 = s1psum2.tile([128, TILE_W], F32, name="ps2")
                nc.tensor.matmul(ps2[:, :cw], lhsT=wi_sb[:], rhs=x1b[:, :cw],
                                 start=True, stop=True)
                nc.vector.tensor_scalar(out=xa[:, c0:c0 + cw],
                                        in0=ps2[:, :cw],
                                        scalar1=bi_sb[:, :1], scalar2=None,
                                        op0=mybir.AluOpType.add)
                leaky_inplace(xa[:, c0:c0 + cw])
                for wi_ in range(c0 // 128, (c0 + cw + 127) // 128):
                    transpose_to_nat(xa[:, wi_ * 128:(wi_ + 1) * 128], wi_,
                                     s1nat, s1tp, ag1_in)
                # fire sub-collectives 0,1 as soon as produced; 2,3 are
                # deferred into layer 1's preamble so their issue does not
                # block the gather stream on the gpsimd queue
                for j in range(2):
                    if c0 < (j + 1) * quart <= c0 + cw:
                        emit_subag(ag1_in, xf1, j)

        # ---- RGCN layers -------------------------------------------------
        # pair after which each next-layer sub-collective can fire:
        # sub-ag j needs nat rows [j*quart, (j+1)*quart)
        subag_after = {((j + 1) * quart + 255) // 256 - 1: j for j in range(2)}

        def rgcn_layer(xin, xout, xfs, ag_next, xfs_next, pre_subags):
            with tc.tile_pool(name="stagp", bufs=14) as stagp, \
                 tc.tile_pool(name="mp", bufs=12) as mp, \
                 tc.tile_pool(name="aggp", bufs=4, space="PSUM") as aggp, \
                 tc.tile_pool(name="meanp", bufs=3) as meanp, \
                 tc.tile_pool(name="cip", bufs=3) as cip, \
                 tc.tile_pool(name="trp", bufs=2, space="PSUM") as trp, \
                 tc.tile_pool(name="tpp", bufs=2, space="PSUM") as tpp, \
                 tc.tile_pool(name="natp", bufs=3) as natp:
                stag_tiles = {}

                def emit_gather(ci):
                    s, lo, n = calls[ci]
                    st = stagp.tile([128, MAX_CALL * 128], GDT, name="st")
                    stag_tiles[ci] = st
                    n_i = n * 128
                    nc.gpsimd.dma_gather(
                        out_ap=st[:, :n_i].rearrange("p (c d) -> p c d",
                                                     d=128),
                        in_ap=xfs[s * seg_rows + base:(s + 1) * seg_rows, :],
                        idxs_ap=idx_sb[:, lo * 8: lo * 8 + n_i // 16],
                        num_idxs=n_i,
                        num_idxs_reg=n_i,
                        elem_size=128,
                    )

                def emit_tail(p, pagg, ci_t):
                    # pair tail, emitted one pair late (software pipeline):
                    # mean (DVE), relation transform (PE), bias (ScalarE),
                    # transpose-to-nat (PE + ScalarE), next-layer sub-ags
                    mean = meanp.tile([128, 512], F32, name="mean")
                    nc.vector.tensor_tensor(out=mean[:], in0=pagg[:],
                                            in1=ci_t[:],
                                            op=mybir.AluOpType.mult)
                    ptr = trp.tile([128, 256], F32, name="ptr")
                    for wh in range(2):
                        w = p * 2 + wh
                        if w >= nw:
                            nc.vector.memset(ptr[:, wh * 128:(wh + 1) * 128],
                                             0.0)
                            continue
                        po = ptr[:, wh * 128:(wh + 1) * 128]
                        nc.tensor.matmul(
                            po, lhsT=wr0_sb[:],
                            rhs=mean[:, wh * 256:wh * 256 + 128],
                            start=True, stop=False)
                        nc.tensor.matmul(
                            po, lhsT=wr1_sb[:],
                            rhs=mean[:, wh * 256 + 128:wh * 256 + 256],
                            start=False, stop=False)
                        nc.tensor.matmul(
                            po, lhsT=wroot_sb[:],
                            rhs=xin[:, w * 128:(w + 1) * 128],
                            start=False, stop=True)
                    psz = min(256, npad - p * 256)
                    nc.scalar.activation(
                        out=xout[:, p * 256:p * 256 + psz],
                        in_=ptr[:, :psz],
                        func=mybir.ActivationFunctionType.Identity,
                        bias=brgcn_sb[:, :1])
                    if ag_next is not None:
                        for wh in range(2):
                            w = p * 2 + wh
                            if w >= nw:
                                continue
                            transpose_to_nat(xout[:, w * 128:(w + 1) * 128],
                                             w, natp, tpp, ag_next)
                        if p in subag_after:
                            emit_subag(ag_next, xfs_next, subag_after[p])

                pending = None
                for p in range(npairs):
                    psl = pair_slots[p]
                    if p == 0:
                        # lead seg-0 calls first, then the deferred previous
                        # sub-collectives (their data is already emitted),
                        # then the just-in-time seg-1 calls
                        for ci_ in prep["sched"][0]:
                            if calls[ci_][0] == 0:
                                emit_gather(ci_)
                        for fn in pre_subags:
                            fn()
                        for ci_ in prep["sched"][0]:
                            if calls[ci_][0] == 1:
                                emit_gather(ci_)
                    else:
                        for ci_ in prep["sched"][p]:
                            emit_gather(ci_)
                    ci_t = cip.tile([128, 512], F32, name="ci")
                    nc.sync.dma_start(out=ci_t[:],
                                      in_=cntinv_d[:, p * 512:(p + 1) * 512])
                    pagg = aggp.tile([128, 512], F32, name="pagg")
                    nmm = len(psl)
                    for i, (sl, s) in enumerate(psl):
                        ci = prep["slot_call"][sl]
                        col = prep["slot_ccol"][sl]
                        st = stag_tiles[ci]
                        m = mp.tile([128, 512], GDT, name="m")
                        nc.vector.tensor_scalar(
                            out=m[:], in0=iota_sb[:],
                            scalar1=keym_sb[:, sl:sl + 1], scalar2=None,
                            op0=mybir.AluOpType.is_equal)
                        nc.tensor.matmul(
                            pagg[:],
                            lhsT=st[:, col * 128:(col + 1) * 128],
                            rhs=m[:],
                            start=(i == 0), stop=(i == nmm - 1))
                    if pending is not None:
                        emit_tail(*pending)
                    pending = (p, pagg, ci_t)
                emit_tail(*pending)

        rgcn_layer(xa, xb, xf1, ag2_in, xf2,
                   pre_subags=[lambda: emit_subag(ag1_in, xf1, 2),
                               lambda: emit_subag(ag1_in, xf1, 3)])
        rgcn_layer(xb, xa, xf2, None, None,
                   pre_subags=[lambda: emit_subag(ag2_in, xf2, 2),
                               lambda: emit_subag(ag2_in, xf2, 3)])

        # ---- head: leaky(x @ Wo + bo) @ Wout + bout ---------------------
        with tc.tile_pool(name="hps", bufs=2, space="PSUM") as hps, \
             tc.tile_pool(name="hps2", bufs=2, space="PSUM") as hps2, \
             tc.tile_pool(name="hbuf", bufs=3) as hbuf:
            for t in range(NT):
                c0 = t * TILE_W
                cw = min(TILE_W, npc - c0)
                psh = hps.tile([128, TILE_W], F32, name="psh")
                nc.tensor.matmul(psh[:, :cw], lhsT=wo_sb[:],
                                 rhs=xa[:, c0:c0 + cw], start=True, stop=True)
                hb = hbuf.tile([128, TILE_W], F32, name="hb")
                nc.scalar.activation(out=hb[:, :cw], in_=psh[:, :cw],
                                     func=mybir.ActivationFunctionType.Lrelu,
                                     bias=bo_sb[:, :1], alpha=SLOPE)
                pso = hps2.tile([2, TILE_W], F32, name="pso")
                nc.tensor.matmul(pso[:, :cw], lhsT=wout_sb[:],
                                 rhs=hb[:, :cw], start=True, stop=True)
                ob = hbuf.tile([2, TILE_W], F32, name="ob")
                nc.vector.tensor_scalar(out=ob[:, :cw], in0=pso[:, :cw],
                                        scalar1=bout_sb[:, :1], scalar2=None,
                                        op0=mybir.AluOpType.add)
                nc.sync.dma_start(out=outT_d[:, c0:c0 + cw], in_=ob[:, :cw])

    nc.compile()
    return nc


# ---------------------------------------------------------------------------
# Public entry point
# ---------------------------------------------------------------------------

_CACHE = {}
GATHER_DTYPE = "bfloat16"


def _get_module(N, T, prep, gather_dtype=None):
    if gather_dtype is None:
        gather_dtype = GATHER_DTYPE
    key = (N, T, prep["npc"], prep["nslots"], tuple(prep["kq"].reshape(-1)),
           gather_dtype)
    if key not in _CACHE:
        _CACHE[key] = _build_module(N, T, prep, gather_dtype=gather_dtype)
    return _CACHE[key]


def _make_in_maps(tweet, prep, Wt, bt, Wi, bi, Wrel, Wroot, brgcn, Wo, bo,
                  Wout, bout, n_cores=N_CORES, gather_dtype=None):
    import ml_dtypes
    if gather_dtype is None:
        gather_dtype = GATHER_DTYPE
    gdt = np.float32 if gather_dtype == "float32" else ml_dtypes.bfloat16
    npc = prep["npc"]
    f32 = np.float32
    shared = dict(
        Wt=np.ascontiguousarray(np.asarray(Wt, f32).astype(gdt)),
        Wi=np.ascontiguousarray(Wi, f32),
        Wr0=np.ascontiguousarray(Wrel[0], f32),
        Wr1=np.ascontiguousarray(Wrel[1], f32),
        Wroot=np.ascontiguousarray(Wroot, f32),
        Wo=np.ascontiguousarray(Wo, f32),
        Wout=np.ascontiguousarray(Wout, f32),
        bt=np.ascontiguousarray(np.reshape(bt, (-1, 1)), f32),
        bi=np.ascontiguousarray(np.reshape(bi, (-1, 1)), f32),
        brgcn=np.ascontiguousarray(np.reshape(brgcn, (-1, 1)), f32),
        bo=np.ascontiguousarray(np.reshape(bo, (-1, 1)), f32),
        bout=np.ascontiguousarray(np.reshape(bout, (-1, 1)), f32),
    )
    in_maps = []
    for c in range(n_cores):
        m = dict(shared)
        m["tweetT"] = np.ascontiguousarray(
            tweet[c * npc:(c + 1) * npc].T.astype(gdt))
        m["idx16"] = np.ascontiguousarray(prep["idx16"][c])
        m["keym"] = np.ascontiguousarray(prep["keym"][c])
        m["cntinv"] = np.ascontiguousarray(prep["cntinv"][c])
        in_maps.append(m)
    return in_maps


def kernel(tweet, edge_index, edge_type, Wt, bt, Wi, bi, Wrel, Wroot, brgcn,
           Wo, bo, Wout, bout):
    tweet = np.asarray(tweet, dtype=np.float32)
    N, T = tweet.shape
    prep = _preprocess(edge_index, edge_type, N)
    nc = _get_module(N, T, prep)
    in_maps = _make_in_maps(tweet, prep, Wt, bt, Wi, bi, Wrel, Wroot, brgcn,
                            Wo, bo, Wout, bout)
    res = bass_utils.run_bass_kernel_spmd(
        nc, in_maps, core_ids=list(range(N_CORES)))
    out = np.concatenate(
        [res.results[c]["outT"].T for c in range(N_CORES)], axis=0)
    return np.ascontiguousarray(out, dtype=np.float32)
